# revision 8
# baseline (speedup 1.0000x reference)
"""Trainium2 8-core GATv2 message-passing kernel (nn_AtomGraphEncoder).

Design (v3 — PE-onehot, quarter-split pipelined AllGather):
- Nodes block-sharded 8x12500, degree-balanced permutation into 98 windows
  of 128 nodes per core; edges assigned to dst's core.
- fs table split into 4 quarter tensors (one per 3125-row slice of every
  core); each quarter AllGathered as soon as projection covers it, so the
  collective pipelines behind the projection and the first edge gathers.
- Per layer, per group of 6 windows: dma_gather fs rows (the only
  GPSIMD-heavy op), expand fd per edge-slot via one-hot matmuls (PE),
  alpha-folded prelu (ACT) + head reduces (DVE) for GATv2 logits, exp,
  msg = ex*fs, segment-sum aggregation via transposed one-hot matmuls
  into per-window PSUM (PE) — no dma_scatter_add, no HBM accumulator.
- |a| and the 0.2 lrelu factor folded into W columns on host
  (pos: a, neg: -0.2|a| with alpha=5 prelu), undone on output.
"""
import sys
import os

import numpy as np
import ml_dtypes

sys.path.insert(0, '/opt/trn_rl_repo')

N, E = 100000, 400000
ATOM_DIM, HID, LAYERS, HEADS = 74, 256, 3, 4
OUT = HID // HEADS
NCORES = 8
NPC = N // NCORES            # 12500
NPQ = NPC // 4               # 3125 rows per core per quarter
QTR = NPQ * NCORES           # 25000 rows per quarter table
CLS = 4
NWIN = (NPC + 127) // 128    # 98
G = 6
NGRP = (NWIN + G - 1) // G   # 17
BF = ml_dtypes.bfloat16
EPS = 1e-20
ALPHA = 0.2

# ---------------------------------------------------------------------------
# host prep


def _fold_weights(W_src, b_src, W_dst, b_dst, attn, bias):
    Ts, Tinvs = [], []
    pos_cnt = np.zeros((LAYERS, HEADS), np.int64)
    zero_cnt = np.zeros((LAYERS, HEADS), np.int64)
    for l in range(LAYERS):
        Tl = np.zeros((HID, HID), np.float64)
        Tinv = np.zeros((HID, HID), np.float64)
        for h in range(HEADS):
            a = np.asarray(attn)[l, h].astype(np.float64)
            order = np.concatenate([
                np.where(a > 0)[0], np.where(a == 0)[0], np.where(a < 0)[0]])
            pos_cnt[l, h] = (a > 0).sum()
            zero_cnt[l, h] = (a == 0).sum()
            for j, p in enumerate(order):
                if a[p] > 0:
                    s = a[p]
                elif a[p] == 0:
                    s = 1.0
                else:
                    s = -ALPHA * abs(a[p])
                Tl[h * OUT + p, h * OUT + j] = s
                Tinv[h * OUT + j, h * OUT + p] = 1.0 / s
        Ts.append(Tl)
        Tinvs.append(Tinv)
    Ws_eff, Wd_eff, bs_eff, bd_eff = [], [], [], []
    for l in range(LAYERS):
        Tp = np.eye(HID) if l == 0 else Tinvs[l - 1]
        Ws = np.asarray(W_src)[l].astype(np.float64)
        Wd = np.asarray(W_dst)[l].astype(np.float64)
        bprev = np.zeros(HID) if l == 0 else np.asarray(bias)[l - 1].astype(np.float64)
        Ws_eff.append((Tp @ Ws @ Ts[l]).astype(np.float32))
        Wd_eff.append((Tp @ Wd @ Ts[l]).astype(np.float32))
        bs_eff.append(((np.asarray(b_src)[l] + bprev @ Ws) @ Ts[l]).astype(np.float32))
        bd_eff.append(((np.asarray(b_dst)[l] + bprev @ Wd) @ Ts[l]).astype(np.float32))
    return Ws_eff, Wd_eff, bs_eff, bd_eff, pos_cnt, zero_cnt, Tinvs[-1]


def _balanced_perm(deg):
    caps = np.full(NWIN, 128, np.int64)
    caps[-1] = NPC - 128 * (NWIN - 1)
    order = np.argsort(-deg, kind="stable")
    fill = np.zeros(NWIN, np.int64)
    r_of_node = np.empty(NPC, np.int64)
    seq = np.concatenate([np.arange(NWIN), np.arange(NWIN)[::-1]])
    ptr = 0
    for node in order:
        while fill[seq[ptr % (2 * NWIN)]] >= caps[seq[ptr % (2 * NWIN)]]:
            ptr += 1
        w = seq[ptr % (2 * NWIN)]
        r_of_node[node] = w * 128 + fill[w]
        fill[w] += 1
        ptr += 1
    nodes_by_r = np.empty(NPC, np.int64)
    nodes_by_r[r_of_node] = np.arange(NPC)
    return r_of_node, nodes_by_r


def _prep(src, dst):
    src = np.asarray(src).astype(np.int64)
    dst = np.asarray(dst).astype(np.int64)

    cores_edges = []
    r_of_node_all = []
    nodes_by_r_all = []
    for c in range(NCORES):
        m = (dst >= c * NPC) & (dst < (c + 1) * NPC)
        es, dl = src[m], dst[m] - c * NPC
        deg = np.bincount(dl, minlength=NPC)
        r_of_node, nodes_by_r = _balanced_perm(deg)
        cores_edges.append((es, r_of_node[dl]))
        r_of_node_all.append(r_of_node)
        nodes_by_r_all.append(nodes_by_r)

    # quarter table row for each absolute node id:
    # class = r//NPQ, idx in table = core*NPQ + r%NPQ
    fscls = np.empty(N, np.int64)
    fsidx = np.empty(N, np.int64)
    for c in range(NCORES):
        r = r_of_node_all[c]
        fscls[c * NPC:(c + 1) * NPC] = r // NPQ
        fsidx[c * NPC:(c + 1) * NPC] = c * NPQ + (r % NPQ)

    cnt = np.zeros((NCORES, NWIN, CLS), np.int64)
    core_wk = []
    for c in range(NCORES):
        es, r_d = cores_edges[c]
        w = r_d // 128
        k = fscls[es]
        np.add.at(cnt[c], (w, k), 1)
        core_wk.append((w, k))
    seg = cnt.max(axis=0)

    run_base = np.zeros((NWIN, CLS), np.int64)
    pieces = [[] for _ in range(NGRP)]
    group_cols = []
    off = 0
    for g in range(NGRP):
        wins = list(range(g * G, min(NWIN, g * G + G)))
        g0 = off
        for k in range(CLS):
            cstart = off
            for w in wins:
                run_base[w, k] = off
                off += seg[w, k]
            off += (-(off - cstart)) % 128
            if off > cstart:
                pieces[g].append((k, cstart, off - cstart))
        group_cols.append((g0 // 128, off // 128))
    S = off
    T_all = S // 128

    pairs = [[] for _ in range(NGRP)]
    PM = np.full((T_all, G), -1, np.int64)
    pi = 0
    for g in range(NGRP):
        wins = list(range(g * G, min(NWIN, g * G + G)))
        c0, c1 = group_cols[g]
        plist = []
        for col in range(c0, c1):
            a, b = col * 128, (col + 1) * 128
            for wl, w in enumerate(wins):
                for k in range(CLS):
                    lo = max(run_base[w, k], a)
                    hi = min(run_base[w, k] + seg[w, k], b)
                    if lo < hi:
                        plist.append((col - c0, wl))
                        break
        seen = set()
        plist2 = []
        for p in plist:
            if p not in seen:
                seen.add(p)
                plist2.append(p)
        wl_first, wl_last, col_first, col_last = {}, {}, {}, {}
        for i, (cl, wl) in enumerate(plist2):
            wl_first.setdefault(wl, i)
            wl_last[wl] = i
            col_first.setdefault(cl, i)
            col_last[cl] = i
        for i, (cl, wl) in enumerate(plist2):
            pairs[g].append(dict(
                c=cl, w=wl, pi=pi,
                e_start=(col_first[cl] == i), e_stop=(col_last[cl] == i),
                a_start=(wl_first[wl] == i), a_stop=(wl_last[wl] == i)))
            PM[c0 + cl, wl] = pi
            pi += 1
        for w in range(len(wins)):
            assert w in wl_first, f"window {g * G + w} has no pairs"
    P_total = pi
    Pmax = max(len(p) for p in pairs)
    Cmax = max(c1 - c0 for c0, c1 in group_cols)

    cores = []
    for c in range(NCORES):
        es, r_d = cores_edges[c]
        w, k = core_wk[c]
        order = np.lexsort((r_d, k, w))
        es_s, rd_s, w_s, k_s = es[order], r_d[order], w[order], k[order]
        key = w_s * CLS + k_s
        uniq, starts = np.unique(key, return_index=True)
        rank = np.arange(len(key)) - np.repeat(
            starts, np.diff(np.concatenate([starts, [len(key)]])))
        slot = run_base[w_s, k_s] + rank
        fs_idx = np.zeros(S, np.int64)
        fs_idx[slot] = fsidx[es_s]
        g_s = w_s // G
        wl_s = w_s - g_s * G
        pi_e = PM[slot // 128, wl_s]
        assert (pi_e >= 0).all()
        ohA = np.zeros((128, P_total, 128), BF)
        ohE = np.zeros((128, P_total, 128), BF)
        ohA[slot % 128, pi_e, rd_s % 128] = 1.0
        ohE[rd_s % 128, pi_e, slot % 128] = 1.0
        cores.append(dict(fs_idx=fs_idx, ohA=ohA, ohE=ohE))

    return dict(seg=seg, run_base=run_base, pieces=pieces, pairs=pairs,
                group_cols=group_cols, S=S, T_all=T_all, P_total=P_total,
                Pmax=Pmax, Cmax=Cmax, cores=cores,
                nodes_by_r=nodes_by_r_all)


def _bf(x):
    return np.asarray(x).astype(BF)


def _wrap16(idx):
    w = np.ascontiguousarray(np.asarray(idx).reshape(-1, 16).T).astype(np.int16)
    return np.tile(w, (8, 1))


# ---------------------------------------------------------------------------
# bass build


def _build(P, pos_cnt):
    import concourse.bass as bass
    import concourse.tile as tile
    from concourse import bacc, mybir, library_config

    S = P['S']
    Pmax, Cmax, P_total = P['Pmax'], P['Cmax'], P['P_total']

    nc = bacc.Bacc("TRN2", target_bir_lowering=False, debug=False,
                   num_devices=NCORES)
    dt = mybir.dt
    atomT_d = nc.dram_tensor("atomT", [ATOM_DIM + 1, NPC], dt.bfloat16,
                             kind="ExternalInput")
    win_d = nc.dram_tensor("win", [ATOM_DIM + 1, HID], dt.bfloat16,
                           kind="ExternalInput")
    wsd_d = nc.dram_tensor("wsd", [128, 2 * LAYERS, 512], dt.bfloat16,
                           kind="ExternalInput")
    fsi_d = nc.dram_tensor("fsi", [128, S // 16], dt.int16, kind="ExternalInput")
    ohE_d = nc.dram_tensor("ohE", [128, P_total, 128], dt.bfloat16,
                           kind="ExternalInput")
    ohA_d = nc.dram_tensor("ohA", [128, P_total, 128], dt.bfloat16,
                           kind="ExternalInput")
    out_d = nc.dram_tensor("out", [NPC, HID], dt.float32, kind="ExternalOutput")

    fs_bounce = nc.dram_tensor("fs_bounce", [NPC, HID], dt.bfloat16)
    fs_q = [nc.dram_tensor(f"fs_q{k}", [QTR, HID], dt.bfloat16,
                           addr_space="Shared") for k in range(CLS)]

    LASTN = NPC - 128 * (NWIN - 1)   # 84
    # quarter q is ready after proj block ceil(NPQ*(q+1)/128)-1
    qlast = [(NPQ * (q + 1) + 127) // 128 - 1 for q in range(CLS)]

    with tile.TileContext(nc) as tc:
        nc.gpsimd.load_library(library_config.mlp)
        with tc.tile_pool(name="persist", bufs=1) as pp, \
             tc.tile_pool(name="atp", bufs=2) as ap_, \
             tc.tile_pool(name="htp", bufs=3) as hp, \
             tc.tile_pool(name="stage", bufs=2) as sp, \
             tc.tile_pool(name="ohp", bufs=2) as op_, \
             tc.tile_pool(name="fsgp", bufs=2) as wp, \
             tc.tile_pool(name="upp", bufs=2) as up_, \
             tc.tile_pool(name="payp", bufs=2) as yp, \
             tc.tile_pool(name="smallp", bufs=2) as mp, \
             tc.tile_pool(name="psE", bufs=2, space="PSUM") as psE, \
             tc.tile_pool(name="aggp", bufs=1, space="PSUM") as aggp:

            fsi = pp.tile([128, S // 16], dt.int16, tag="fsi")
            wsd = pp.tile([128, 2 * LAYERS, 512], dt.bfloat16, tag="wsd")
            win = pp.tile([ATOM_DIM + 1, HID], dt.bfloat16, tag="win")
            fdh = pp.tile([128, NWIN, HID], dt.bfloat16, tag="fdh")
            nc.sync.dma_start(fsi[:], fsi_d[:])
            nc.sync.dma_start(wsd[:], wsd_d[:])
            nc.sync.dma_start(win[:], win_d[:])

            # ---- input projection: fdh = h0 (node-major, permuted order)
            nc.vector.memset(fdh[:, NWIN - 1, :], 0.0)
            for a in range(NWIN):
                nt = 128 if a < NWIN - 1 else LASTN
                at = ap_.tile([ATOM_DIM + 1, 128], dt.bfloat16, tag="at")
                nc.sync.dma_start(at[:, 0:nt], atomT_d[:, a * 128:a * 128 + nt])
                ps = psE.tile([128, 512], dt.float32, tag="pse")
                nc.tensor.matmul(ps[0:nt, 0:HID], at[:, 0:nt], win[:],
                                 start=True, stop=True)
                nc.scalar.activation(out=fdh[0:nt, a, :], in_=ps[0:nt, 0:HID],
                                     func=mybir.ActivationFunctionType.Copy)

            for l in range(LAYERS):
                last = l == LAYERS - 1
                # ---- projection phase: fs -> fs_bounce (HBM), fd -> fdh
                for a in range(NWIN):
                    nt = 128 if a < NWIN - 1 else LASTN
                    hTst = hp.tile([128, 2, 128], dt.bfloat16, tag="hTst")
                    for cch in range(2):
                        nc.sync.dma_start(
                            hTst[:, cch, :],
                            fdh[:, a, cch * 128:(cch + 1) * 128],
                            transpose=True)
                    ps = psE.tile([128, 512], dt.float32, tag="pse")
                    for kc in range(2):
                        nc.tensor.matmul(
                            ps[0:nt, :], hTst[:, kc, 0:nt],
                            wsd[:, l * 2 + kc, :],
                            start=(kc == 0), stop=(kc == 1))
                    j = a % 8
                    if j == 0:
                        fs_sb = sp.tile([128, 8, HID], dt.bfloat16, tag="fs_sb")
                    nc.scalar.activation(out=fs_sb[0:nt, j, :], in_=ps[0:nt, 0:HID],
                                         func=mybir.ActivationFunctionType.Copy)
                    nc.vector.tensor_copy(out=fdh[0:nt, a, :],
                                          in_=ps[0:nt, HID:512])
                    if j == 7 or a == NWIN - 1:
                        a0 = a - j
                        fullc = j + (1 if nt == 128 else 0)
                        if fullc:
                            nc.sync.dma_start(
                                fs_bounce[a0 * 128:(a0 + fullc) * 128, :].rearrange(
                                    "(a p) e -> p a e", p=128),
                                fs_sb[:, 0:fullc, :])
                        if nt < 128:
                            nc.sync.dma_start(
                                fs_bounce[(NWIN - 1) * 128:NPC, :],
                                fs_sb[0:nt, j, :])
                        # fire quarter AllGathers as projection covers them
                        for q in range(CLS):
                            if qlast[q] <= a and qlast[q] > a - 8:
                                nc.gpsimd.collective_compute(
                                    "AllGather", mybir.AluOpType.bypass,
                                    replica_groups=[list(range(NCORES))],
                                    ins=[fs_bounce[q * NPQ:(q + 1) * NPQ, :].opt()],
                                    outs=[fs_q[q][:].opt()])

                # ---- edge phase: software-pipelined groups
                holdover = None
                for g in range(NGRP):
                    c0, c1 = P['group_cols'][g]
                    Cg = c1 - c0
                    wins = list(range(g * G, min(NWIN, g * G + G)))
                    pg = P['pairs'][g]
                    Pg = len(pg)
                    pbase = pg[0]['pi']

                    ohEt = op_.tile([128, Pmax, 128], dt.bfloat16, tag="ohE")
                    ohAt = op_.tile([128, Pmax, 128], dt.bfloat16, tag="ohA")
                    nc.sync.dma_start(ohEt[:, 0:Pg, :],
                                      ohE_d[:, pbase:pbase + Pg, :])
                    nc.sync.dma_start(ohAt[:, 0:Pg, :],
                                      ohA_d[:, pbase:pbase + Pg, :])
                    fsg = wp.tile([128, Cmax, HID], dt.bfloat16, tag="fsg")
                    for (k, soff, n) in P['pieces'][g]:
                        nc.gpsimd.dma_gather(
                            fsg[:, soff // 128 - c0:(soff + n) // 128 - c0, :],
                            fs_q[k][:],
                            fsi[:, soff // 16:(soff + n) // 16], n, n, HID)

                    # expand fd per slot-col (2 cols per PSUM tile) + u = fs+fd
                    upre = up_.tile([128, Cmax, HID], dt.bfloat16, tag="upre", bufs=1)
                    percol = {}
                    for i, pr in enumerate(pg):
                        percol.setdefault(pr['c'], []).append(i)
                    for cl0 in range(0, Cg, 2):
                        ncol = min(2, Cg - cl0)
                        ps = psE.tile([128, 512], dt.float32, tag="pse")
                        for dc in range(ncol):
                            cl = cl0 + dc
                            idxs = percol[cl]
                            for ii, i in enumerate(idxs):
                                pr = pg[i]
                                nc.tensor.matmul(
                                    ps[:, dc * HID:(dc + 1) * HID],
                                    ohEt[:, i, :], fdh[:, wins[pr['w']], :],
                                    start=(ii == 0), stop=(ii == len(idxs) - 1))
                        nc.vector.tensor_tensor(
                            out=upre[:, cl0:cl0 + ncol, :],
                            in0=ps[:, 0:ncol * HID].rearrange(
                                "p (t e) -> p t e", t=ncol),
                            in1=fsg[:, cl0:cl0 + ncol, :],
                            op=mybir.AluOpType.add)

                    # alpha-folded prelu + per-head logits
                    for h in range(HEADS):
                        kp = int(pos_cnt[l, h])
                        if kp:
                            nc.scalar.activation(
                                out=upre[:, 0:Cg, h * OUT:h * OUT + kp],
                                in_=upre[:, 0:Cg, h * OUT:h * OUT + kp],
                                func=mybir.ActivationFunctionType.Prelu,
                                alpha=ALPHA)
                        if kp < OUT:
                            nc.scalar.activation(
                                out=upre[:, 0:Cg, h * OUT + kp:(h + 1) * OUT],
                                in_=upre[:, 0:Cg, h * OUT + kp:(h + 1) * OUT],
                                func=mybir.ActivationFunctionType.Prelu,
                                alpha=1.0 / ALPHA)
                    lg = mp.tile([128, Cmax, 4], dt.float32, tag="lg")
                    for h in range(HEADS):
                        nc.vector.tensor_reduce(
                            out=lg[:, 0:Cg, h],
                            in_=upre[:, 0:Cg, h * OUT:(h + 1) * OUT],
                            axis=mybir.AxisListType.X, op=mybir.AluOpType.add)
                    pay = yp.tile([128, Cmax, HID + 4], dt.bfloat16, tag="pay")
                    nc.scalar.activation(out=pay[:, 0:Cg, HID:HID + 4],
                                         in_=lg[:, 0:Cg, :],
                                         func=mybir.ActivationFunctionType.Exp)
                    nc.vector.tensor_tensor(
                        out=pay[:, 0:Cg, 0:HID].rearrange(
                            "p t (h d) -> p t h d", h=HEADS),
                        in0=fsg[:, 0:Cg, :].rearrange(
                            "p t (h d) -> p t h d", h=HEADS),
                        in1=pay[:, 0:Cg, HID:HID + 4].unsqueeze(3).broadcast_to(
                            [128, Cg, HEADS, OUT]),
                        op=mybir.AluOpType.mult)

                    if holdover is not None:
                        _agg_norm(nc, mybir, aggp, mp, sp, fdh, out_d,
                                  holdover, last)
                    holdover = (g, wins, pg, ohAt, pay)
                if holdover is not None:
                    _agg_norm(nc, mybir, aggp, mp, sp, fdh, out_d,
                              holdover, last)
    nc.compile()
    return nc


def _agg_norm(nc, mybir, aggp, mp, sp, fdh, out_d, holdover, last):
    dt = mybir.dt
    g, wins, pg, ohAt, pay = holdover
    cur = {}
    outst = None
    if last:
        outst = sp.tile([128, G, HID], dt.float32, tag="outst", bufs=1)
    for i, pr in enumerate(pg):
        wl = pr['w']
        if pr['a_start']:
            cur[wl] = aggp.tile([128, HID + 4], dt.float32, tag=f"agg{wl}",
                                name=f"aggps{wl}")
        psA = cur[wl]
        nc.tensor.matmul(psA[:, 0:HID + 4], ohAt[:, i, :],
                         pay[:, pr['c'], 0:HID + 4],
                         start=pr['a_start'], stop=pr['a_stop'])
        if pr['a_stop']:
            W = wins[wl]
            denf = mp.tile([128, 4], dt.float32, tag="denf")
            rec = mp.tile([128, 4], dt.float32, tag="rec")
            nc.scalar.activation(out=denf[:], in_=psA[:, HID:HID + 4],
                                 func=mybir.ActivationFunctionType.Copy,
                                 bias=EPS)
            nc.vector.reciprocal(out=rec[:], in_=denf[:])
            for h in range(HEADS):
                dst = (outst[:, wl, h * OUT:(h + 1) * OUT] if last
                       else fdh[:, W, h * OUT:(h + 1) * OUT])
                nc.scalar.activation(
                    out=dst, in_=psA[:, h * OUT:(h + 1) * OUT],
                    func=mybir.ActivationFunctionType.Copy,
                    scale=rec[:, h:h + 1])
    if last:
        LASTN = NPC - 128 * (NWIN - 1)
        w0 = wins[0]
        fullw = len(wins) if wins[-1] < NWIN - 1 else len(wins) - 1
        if fullw:
            nc.sync.dma_start(
                out_d[w0 * 128:(w0 + fullw) * 128, :].rearrange(
                    "(a p) e -> p a e", p=128),
                outst[:, 0:fullw, :])
        if wins[-1] == NWIN - 1:
            nc.sync.dma_start(
                out_d[(NWIN - 1) * 128:NPC, :],
                outst[0:LASTN, len(wins) - 1, :])


# ---------------------------------------------------------------------------


def kernel(**inputs):
    from concourse.bass_utils import run_bass_kernel_spmd

    src = np.asarray(inputs['src'])
    dst = np.asarray(inputs['dst'])
    atom = np.asarray(inputs['atom_feat']).astype(np.float32)
    Ws_eff, Wd_eff, bs_eff, bd_eff, pos_cnt, zero_cnt, T2inv = _fold_weights(
        inputs['W_src'], inputs['b_src'], inputs['W_dst'], inputs['b_dst'],
        inputs['attn'], inputs['bias'])
    for l in range(LAYERS):
        assert np.abs(bs_eff[l]).max() < 1e-12 and np.abs(bd_eff[l]).max() < 1e-12, \
            "nonzero GAT biases not supported by this kernel build"
    assert (zero_cnt == 0).all(), "zero attention weights not supported"

    P = _prep(src, dst)

    win_np = np.zeros((ATOM_DIM + 1, HID), np.float32)
    win_np[:ATOM_DIM] = np.asarray(inputs['W_in'])
    win_np[ATOM_DIM] = np.asarray(inputs['b_in'])
    wsd_np = np.zeros((128, 2 * LAYERS, 512), np.float32)
    for l in range(LAYERS):
        for kc in range(2):
            wsd_np[:, l * 2 + kc, 0:HID] = Ws_eff[l][kc * 128:(kc + 1) * 128]
            wsd_np[:, l * 2 + kc, HID:512] = Wd_eff[l][kc * 128:(kc + 1) * 128]

    nc = _build(P, pos_cnt)

    in_maps = []
    for c in range(NCORES):
        cd = P['cores'][c]
        at = np.zeros((ATOM_DIM + 1, NPC), np.float32)
        at[:ATOM_DIM] = atom[c * NPC + P['nodes_by_r'][c]].T
        at[ATOM_DIM] = 1.0
        in_maps.append({
            'atomT': _bf(at), 'win': _bf(win_np), 'wsd': _bf(wsd_np),
            'fsi': _wrap16(cd['fs_idx']),
            'ohE': cd['ohE'], 'ohA': cd['ohA'],
        })
    res = run_bass_kernel_spmd(nc, in_maps, core_ids=list(range(NCORES)),
                               trace=bool(os.environ.get('KBT_TRACE')))
    kernel._last = res
    full = np.empty((N, HID), np.float64)
    for c in range(NCORES):
        full[c * NPC + P['nodes_by_r'][c]] = res.results[c]['out']
    full = full @ T2inv + np.asarray(inputs['bias'])[LAYERS - 1][None]
    return full.astype(np.float32)


if __name__ == '__main__':
    import jax
    with jax.default_device(jax.devices('cpu')[0]):
        import reference
        inputs = {k: np.asarray(v) for k, v in reference.setup_inputs().items()}
    got = kernel(**inputs)
    print("kernel out:", got.shape, got.dtype, np.abs(got).mean())


# revision 9
# speedup vs baseline: 1.2944x; 1.2944x over previous
"""Trainium2 8-core GATv2 message-passing kernel (nn_AtomGraphEncoder).

Design (v3 — PE-onehot, quarter-split pipelined AllGather):
- Nodes block-sharded 8x12500, degree-balanced permutation into 98 windows
  of 128 nodes per core; edges assigned to dst's core.
- fs table split into 4 quarter tensors (one per 3125-row slice of every
  core); each quarter AllGathered as soon as projection covers it, so the
  collective pipelines behind the projection and the first edge gathers.
- Per layer, per group of 6 windows: dma_gather fs rows (the only
  GPSIMD-heavy op), expand fd per edge-slot via one-hot matmuls (PE),
  alpha-folded prelu (ACT) + head reduces (DVE) for GATv2 logits, exp,
  msg = ex*fs, segment-sum aggregation via transposed one-hot matmuls
  into per-window PSUM (PE) — no dma_scatter_add, no HBM accumulator.
- |a| and the 0.2 lrelu factor folded into W columns on host
  (pos: a, neg: -0.2|a| with alpha=5 prelu), undone on output.
"""
import sys
import os

import numpy as np
import ml_dtypes

sys.path.insert(0, '/opt/trn_rl_repo')

N, E = 100000, 400000
ATOM_DIM, HID, LAYERS, HEADS = 74, 256, 3, 4
OUT = HID // HEADS
NCORES = 8
NPC = N // NCORES            # 12500
NPQ = NPC // 4               # 3125 rows per core per quarter
QTR = NPQ * NCORES           # 25000 rows per quarter table
CLS = 4
NWIN = (NPC + 127) // 128    # 98
G = 6
NGRP = (NWIN + G - 1) // G   # 17
BF = ml_dtypes.bfloat16
EPS = 1e-20
ALPHA = 0.2

# ---------------------------------------------------------------------------
# host prep


def _fold_weights(W_src, b_src, W_dst, b_dst, attn, bias):
    Ts, Tinvs = [], []
    pos_cnt = np.zeros((LAYERS, HEADS), np.int64)
    zero_cnt = np.zeros((LAYERS, HEADS), np.int64)
    for l in range(LAYERS):
        Tl = np.zeros((HID, HID), np.float64)
        Tinv = np.zeros((HID, HID), np.float64)
        for h in range(HEADS):
            a = np.asarray(attn)[l, h].astype(np.float64)
            order = np.concatenate([
                np.where(a > 0)[0], np.where(a == 0)[0], np.where(a < 0)[0]])
            pos_cnt[l, h] = (a > 0).sum()
            zero_cnt[l, h] = (a == 0).sum()
            for j, p in enumerate(order):
                if a[p] > 0:
                    s = a[p]
                elif a[p] == 0:
                    s = 1.0
                else:
                    s = -ALPHA * abs(a[p])
                Tl[h * OUT + p, h * OUT + j] = s
                Tinv[h * OUT + j, h * OUT + p] = 1.0 / s
        Ts.append(Tl)
        Tinvs.append(Tinv)
    Ws_eff, Wd_eff, bs_eff, bd_eff = [], [], [], []
    for l in range(LAYERS):
        Tp = np.eye(HID) if l == 0 else Tinvs[l - 1]
        Ws = np.asarray(W_src)[l].astype(np.float64)
        Wd = np.asarray(W_dst)[l].astype(np.float64)
        bprev = np.zeros(HID) if l == 0 else np.asarray(bias)[l - 1].astype(np.float64)
        Ws_eff.append((Tp @ Ws @ Ts[l]).astype(np.float32))
        Wd_eff.append((Tp @ Wd @ Ts[l]).astype(np.float32))
        bs_eff.append(((np.asarray(b_src)[l] + bprev @ Ws) @ Ts[l]).astype(np.float32))
        bd_eff.append(((np.asarray(b_dst)[l] + bprev @ Wd) @ Ts[l]).astype(np.float32))
    return Ws_eff, Wd_eff, bs_eff, bd_eff, pos_cnt, zero_cnt, Tinvs[-1]


def _balanced_perm(deg):
    caps = np.full(NWIN, 128, np.int64)
    caps[-1] = NPC - 128 * (NWIN - 1)
    order = np.argsort(-deg, kind="stable")
    fill = np.zeros(NWIN, np.int64)
    r_of_node = np.empty(NPC, np.int64)
    seq = np.concatenate([np.arange(NWIN), np.arange(NWIN)[::-1]])
    ptr = 0
    for node in order:
        while fill[seq[ptr % (2 * NWIN)]] >= caps[seq[ptr % (2 * NWIN)]]:
            ptr += 1
        w = seq[ptr % (2 * NWIN)]
        r_of_node[node] = w * 128 + fill[w]
        fill[w] += 1
        ptr += 1
    nodes_by_r = np.empty(NPC, np.int64)
    nodes_by_r[r_of_node] = np.arange(NPC)
    return r_of_node, nodes_by_r


def _prep(src, dst):
    src = np.asarray(src).astype(np.int64)
    dst = np.asarray(dst).astype(np.int64)

    cores_edges = []
    r_of_node_all = []
    nodes_by_r_all = []
    for c in range(NCORES):
        m = (dst >= c * NPC) & (dst < (c + 1) * NPC)
        es, dl = src[m], dst[m] - c * NPC
        deg = np.bincount(dl, minlength=NPC)
        r_of_node, nodes_by_r = _balanced_perm(deg)
        cores_edges.append((es, r_of_node[dl]))
        r_of_node_all.append(r_of_node)
        nodes_by_r_all.append(nodes_by_r)

    # quarter table row for each absolute node id:
    # class = r//NPQ, idx in table = core*NPQ + r%NPQ
    fscls = np.empty(N, np.int64)
    fsidx = np.empty(N, np.int64)
    for c in range(NCORES):
        r = r_of_node_all[c]
        fscls[c * NPC:(c + 1) * NPC] = r // NPQ
        fsidx[c * NPC:(c + 1) * NPC] = c * NPQ + (r % NPQ)

    cnt = np.zeros((NCORES, NWIN, CLS), np.int64)
    core_wk = []
    for c in range(NCORES):
        es, r_d = cores_edges[c]
        w = r_d // 128
        k = fscls[es]
        np.add.at(cnt[c], (w, k), 1)
        core_wk.append((w, k))
    seg = cnt.max(axis=0)

    run_base = np.zeros((NWIN, CLS), np.int64)
    pieces = [[] for _ in range(NGRP)]
    group_cols = []
    off = 0
    for g in range(NGRP):
        wins = list(range(g * G, min(NWIN, g * G + G)))
        g0 = off
        for k in range(CLS):
            cstart = off
            for w in wins:
                run_base[w, k] = off
                off += seg[w, k]
            off += (-(off - cstart)) % 128
            if off > cstart:
                pieces[g].append((k, cstart, off - cstart))
        group_cols.append((g0 // 128, off // 128))
    S = off
    T_all = S // 128

    pairs = [[] for _ in range(NGRP)]
    PM = np.full((T_all, G), -1, np.int64)
    pi = 0
    for g in range(NGRP):
        wins = list(range(g * G, min(NWIN, g * G + G)))
        c0, c1 = group_cols[g]
        plist = []
        for col in range(c0, c1):
            a, b = col * 128, (col + 1) * 128
            for wl, w in enumerate(wins):
                for k in range(CLS):
                    lo = max(run_base[w, k], a)
                    hi = min(run_base[w, k] + seg[w, k], b)
                    if lo < hi:
                        plist.append((col - c0, wl))
                        break
        seen = set()
        plist2 = []
        for p in plist:
            if p not in seen:
                seen.add(p)
                plist2.append(p)
        wl_first, wl_last, col_first, col_last = {}, {}, {}, {}
        for i, (cl, wl) in enumerate(plist2):
            wl_first.setdefault(wl, i)
            wl_last[wl] = i
            col_first.setdefault(cl, i)
            col_last[cl] = i
        for i, (cl, wl) in enumerate(plist2):
            pairs[g].append(dict(
                c=cl, w=wl, pi=pi,
                e_start=(col_first[cl] == i), e_stop=(col_last[cl] == i),
                a_start=(wl_first[wl] == i), a_stop=(wl_last[wl] == i)))
            PM[c0 + cl, wl] = pi
            pi += 1
        for w in range(len(wins)):
            assert w in wl_first, f"window {g * G + w} has no pairs"
    P_total = pi
    Pmax = max(len(p) for p in pairs)
    Cmax = max(c1 - c0 for c0, c1 in group_cols)

    cores = []
    for c in range(NCORES):
        es, r_d = cores_edges[c]
        w, k = core_wk[c]
        order = np.lexsort((r_d, k, w))
        es_s, rd_s, w_s, k_s = es[order], r_d[order], w[order], k[order]
        key = w_s * CLS + k_s
        uniq, starts = np.unique(key, return_index=True)
        rank = np.arange(len(key)) - np.repeat(
            starts, np.diff(np.concatenate([starts, [len(key)]])))
        slot = run_base[w_s, k_s] + rank
        fs_idx = np.zeros(S, np.int64)
        fs_idx[slot] = fsidx[es_s]
        g_s = w_s // G
        wl_s = w_s - g_s * G
        pi_e = PM[slot // 128, wl_s]
        assert (pi_e >= 0).all()
        ohA = np.zeros((128, P_total, 128), BF)
        ohE = np.zeros((128, P_total, 128), BF)
        ohA[slot % 128, pi_e, rd_s % 128] = 1.0
        ohE[rd_s % 128, pi_e, slot % 128] = 1.0
        cores.append(dict(fs_idx=fs_idx, ohA=ohA, ohE=ohE))

    return dict(seg=seg, run_base=run_base, pieces=pieces, pairs=pairs,
                group_cols=group_cols, S=S, T_all=T_all, P_total=P_total,
                Pmax=Pmax, Cmax=Cmax, cores=cores,
                nodes_by_r=nodes_by_r_all)


def _bf(x):
    return np.asarray(x).astype(BF)


def _wrap16(idx):
    w = np.ascontiguousarray(np.asarray(idx).reshape(-1, 16).T).astype(np.int16)
    return np.tile(w, (8, 1))


# ---------------------------------------------------------------------------
# bass build


def _build(P, pos_cnt):
    import concourse.bass as bass
    import concourse.tile as tile
    from concourse import bacc, mybir, library_config

    S = P['S']
    Pmax, Cmax, P_total = P['Pmax'], P['Cmax'], P['P_total']

    nc = bacc.Bacc("TRN2", target_bir_lowering=False, debug=False,
                   num_devices=NCORES)
    dt = mybir.dt
    atomT_d = nc.dram_tensor("atomT", [ATOM_DIM + 1, NPC], dt.bfloat16,
                             kind="ExternalInput")
    win_d = nc.dram_tensor("win", [ATOM_DIM + 1, HID], dt.bfloat16,
                           kind="ExternalInput")
    wsd_d = nc.dram_tensor("wsd", [128, 2 * LAYERS, 512], dt.bfloat16,
                           kind="ExternalInput")
    fsi_d = nc.dram_tensor("fsi", [128, S // 16], dt.int16, kind="ExternalInput")
    ohE_d = nc.dram_tensor("ohE", [128, P_total, 128], dt.bfloat16,
                           kind="ExternalInput")
    ohA_d = nc.dram_tensor("ohA", [128, P_total, 128], dt.bfloat16,
                           kind="ExternalInput")
    ident_d = nc.dram_tensor("ident", [128, 128], dt.bfloat16,
                             kind="ExternalInput")
    out_d = nc.dram_tensor("out", [NPC, HID], dt.float32, kind="ExternalOutput")

    fs_bounce = nc.dram_tensor("fs_bounce", [NPC, HID], dt.bfloat16)
    fs_q = [nc.dram_tensor(f"fs_q{k}", [QTR, HID], dt.bfloat16,
                           addr_space="Shared") for k in range(CLS)]

    LASTN = NPC - 128 * (NWIN - 1)   # 84
    # quarter q is ready after proj block ceil(NPQ*(q+1)/128)-1
    qlast = [(NPQ * (q + 1) + 127) // 128 - 1 for q in range(CLS)]

    with tile.TileContext(nc) as tc:
        nc.gpsimd.load_library(library_config.mlp)
        with tc.tile_pool(name="persist", bufs=1) as pp, \
             tc.tile_pool(name="atp", bufs=2) as ap_, \
             tc.tile_pool(name="htp", bufs=3) as hp, \
             tc.tile_pool(name="stage", bufs=2) as sp, \
             tc.tile_pool(name="ohp", bufs=2) as op_, \
             tc.tile_pool(name="fsgp", bufs=2) as wp, \
             tc.tile_pool(name="upp", bufs=2) as up_, \
             tc.tile_pool(name="payp", bufs=2) as yp, \
             tc.tile_pool(name="smallp", bufs=2) as mp, \
             tc.tile_pool(name="psE", bufs=2, space="PSUM") as psE, \
             tc.tile_pool(name="aggp", bufs=1, space="PSUM") as aggp:

            fsi = pp.tile([128, S // 16], dt.int16, tag="fsi")
            wsd = pp.tile([128, 2 * LAYERS, 512], dt.bfloat16, tag="wsd")
            win = pp.tile([ATOM_DIM + 1, HID], dt.bfloat16, tag="win")
            fdh = pp.tile([128, NWIN, HID], dt.bfloat16, tag="fdh")
            ident = pp.tile([128, 128], dt.bfloat16, tag="ident")
            nc.sync.dma_start(ident[:], ident_d[:])
            nc.sync.dma_start(fsi[:], fsi_d[:])
            nc.sync.dma_start(wsd[:], wsd_d[:])
            nc.sync.dma_start(win[:], win_d[:])

            # ---- input projection: fdh = h0 (node-major, permuted order)
            nc.vector.memset(fdh[:, NWIN - 1, :], 0.0)
            for a in range(NWIN):
                nt = 128 if a < NWIN - 1 else LASTN
                at = ap_.tile([ATOM_DIM + 1, 128], dt.bfloat16, tag="at")
                nc.sync.dma_start(at[:, 0:nt], atomT_d[:, a * 128:a * 128 + nt])
                ps = psE.tile([128, 512], dt.float32, tag="pse")
                nc.tensor.matmul(ps[0:nt, 0:HID], at[:, 0:nt], win[:],
                                 start=True, stop=True)
                nc.scalar.activation(out=fdh[0:nt, a, :], in_=ps[0:nt, 0:HID],
                                     func=mybir.ActivationFunctionType.Copy)

            for l in range(LAYERS):
                last = l == LAYERS - 1
                # ---- projection phase: fs -> fs_bounce (HBM), fd -> fdh
                for a in range(NWIN):
                    nt = 128 if a < NWIN - 1 else LASTN
                    hTst = hp.tile([128, 2, 128], dt.bfloat16, tag="hTst")
                    for cch in range(2):
                        tg = cch * 2 + (a & 1)
                        pt = aggp.tile([128, 128], dt.bfloat16,
                                       tag=f"agg{tg}", name=f"tp{tg}")
                        nc.tensor.transpose(
                            pt[:], fdh[:, a, cch * 128:(cch + 1) * 128],
                            ident[:])
                        if cch == 0:
                            nc.scalar.activation(
                                out=hTst[:, cch, :], in_=pt[:],
                                func=mybir.ActivationFunctionType.Copy)
                        else:
                            nc.vector.tensor_copy(out=hTst[:, cch, :],
                                                  in_=pt[:])
                    ps = psE.tile([128, 512], dt.float32, tag="pse")
                    for kc in range(2):
                        nc.tensor.matmul(
                            ps[0:nt, :], hTst[:, kc, 0:nt],
                            wsd[:, l * 2 + kc, :],
                            start=(kc == 0), stop=(kc == 1))
                    j = a % 8
                    if j == 0:
                        fs_sb = sp.tile([128, 8, HID], dt.bfloat16, tag="fs_sb")
                    nc.scalar.activation(out=fs_sb[0:nt, j, :], in_=ps[0:nt, 0:HID],
                                         func=mybir.ActivationFunctionType.Copy)
                    nc.vector.tensor_copy(out=fdh[0:nt, a, :],
                                          in_=ps[0:nt, HID:512])
                    if j == 7 or a == NWIN - 1:
                        a0 = a - j
                        fullc = j + (1 if nt == 128 else 0)
                        if fullc:
                            nc.sync.dma_start(
                                fs_bounce[a0 * 128:(a0 + fullc) * 128, :].rearrange(
                                    "(a p) e -> p a e", p=128),
                                fs_sb[:, 0:fullc, :])
                        if nt < 128:
                            nc.sync.dma_start(
                                fs_bounce[(NWIN - 1) * 128:NPC, :],
                                fs_sb[0:nt, j, :])
                        # fire quarter AllGathers as projection covers them
                        for q in range(CLS):
                            if qlast[q] <= a and qlast[q] > a - 8:
                                nc.gpsimd.collective_compute(
                                    "AllGather", mybir.AluOpType.bypass,
                                    replica_groups=[list(range(NCORES))],
                                    ins=[fs_bounce[q * NPQ:(q + 1) * NPQ, :].opt()],
                                    outs=[fs_q[q][:].opt()])

                # ---- edge phase: software-pipelined groups
                holdover = None
                for g in range(NGRP):
                    c0, c1 = P['group_cols'][g]
                    Cg = c1 - c0
                    wins = list(range(g * G, min(NWIN, g * G + G)))
                    pg = P['pairs'][g]
                    Pg = len(pg)
                    pbase = pg[0]['pi']

                    ohEt = op_.tile([128, Pmax, 128], dt.bfloat16, tag="ohE")
                    ohAt = op_.tile([128, Pmax, 128], dt.bfloat16, tag="ohA")
                    nc.sync.dma_start(ohEt[:, 0:Pg, :],
                                      ohE_d[:, pbase:pbase + Pg, :])
                    nc.sync.dma_start(ohAt[:, 0:Pg, :],
                                      ohA_d[:, pbase:pbase + Pg, :])
                    fsg = wp.tile([128, Cmax, HID], dt.bfloat16, tag="fsg")
                    for (k, soff, n) in P['pieces'][g]:
                        nc.gpsimd.dma_gather(
                            fsg[:, soff // 128 - c0:(soff + n) // 128 - c0, :],
                            fs_q[k][:],
                            fsi[:, soff // 16:(soff + n) // 16], n, n, HID)

                    # expand fd per slot-col (2 cols per PSUM tile) + u = fs+fd
                    upre = up_.tile([128, Cmax, HID], dt.bfloat16, tag="upre", bufs=1)
                    percol = {}
                    for i, pr in enumerate(pg):
                        percol.setdefault(pr['c'], []).append(i)
                    for cl0 in range(0, Cg, 2):
                        ncol = min(2, Cg - cl0)
                        ps = psE.tile([128, 512], dt.float32, tag="pse")
                        for dc in range(ncol):
                            cl = cl0 + dc
                            idxs = percol[cl]
                            for ii, i in enumerate(idxs):
                                pr = pg[i]
                                nc.tensor.matmul(
                                    ps[:, dc * HID:(dc + 1) * HID],
                                    ohEt[:, i, :], fdh[:, wins[pr['w']], :],
                                    start=(ii == 0), stop=(ii == len(idxs) - 1))
                        nc.vector.tensor_tensor(
                            out=upre[:, cl0:cl0 + ncol, :],
                            in0=ps[:, 0:ncol * HID].rearrange(
                                "p (t e) -> p t e", t=ncol),
                            in1=fsg[:, cl0:cl0 + ncol, :],
                            op=mybir.AluOpType.add)

                    # alpha-folded prelu + per-head logits
                    for h in range(HEADS):
                        kp = int(pos_cnt[l, h])
                        if kp:
                            nc.scalar.activation(
                                out=upre[:, 0:Cg, h * OUT:h * OUT + kp],
                                in_=upre[:, 0:Cg, h * OUT:h * OUT + kp],
                                func=mybir.ActivationFunctionType.Prelu,
                                alpha=ALPHA)
                        if kp < OUT:
                            nc.scalar.activation(
                                out=upre[:, 0:Cg, h * OUT + kp:(h + 1) * OUT],
                                in_=upre[:, 0:Cg, h * OUT + kp:(h + 1) * OUT],
                                func=mybir.ActivationFunctionType.Prelu,
                                alpha=1.0 / ALPHA)
                    lg = mp.tile([128, Cmax, 4], dt.float32, tag="lg")
                    for h in range(HEADS):
                        nc.vector.tensor_reduce(
                            out=lg[:, 0:Cg, h],
                            in_=upre[:, 0:Cg, h * OUT:(h + 1) * OUT],
                            axis=mybir.AxisListType.X, op=mybir.AluOpType.add)
                    pay = yp.tile([128, Cmax, HID + 4], dt.bfloat16, tag="pay")
                    nc.scalar.activation(out=pay[:, 0:Cg, HID:HID + 4],
                                         in_=lg[:, 0:Cg, :],
                                         func=mybir.ActivationFunctionType.Exp)
                    nc.vector.tensor_tensor(
                        out=pay[:, 0:Cg, 0:HID].rearrange(
                            "p t (h d) -> p t h d", h=HEADS),
                        in0=fsg[:, 0:Cg, :].rearrange(
                            "p t (h d) -> p t h d", h=HEADS),
                        in1=pay[:, 0:Cg, HID:HID + 4].unsqueeze(3).broadcast_to(
                            [128, Cg, HEADS, OUT]),
                        op=mybir.AluOpType.mult)

                    if holdover is not None:
                        _agg_norm(nc, mybir, aggp, mp, sp, fdh, out_d,
                                  holdover, last)
                    holdover = (g, wins, pg, ohAt, pay)
                if holdover is not None:
                    _agg_norm(nc, mybir, aggp, mp, sp, fdh, out_d,
                              holdover, last)
    nc.compile()
    return nc


def _agg_norm(nc, mybir, aggp, mp, sp, fdh, out_d, holdover, last):
    dt = mybir.dt
    g, wins, pg, ohAt, pay = holdover
    cur = {}
    outst = None
    if last:
        outst = sp.tile([128, G, HID], dt.float32, tag="outst", bufs=1)
    for i, pr in enumerate(pg):
        wl = pr['w']
        if pr['a_start']:
            cur[wl] = aggp.tile([128, HID + 4], dt.float32, tag=f"agg{wl}",
                                name=f"aggps{wl}")
        psA = cur[wl]
        nc.tensor.matmul(psA[:, 0:HID + 4], ohAt[:, i, :],
                         pay[:, pr['c'], 0:HID + 4],
                         start=pr['a_start'], stop=pr['a_stop'])
        if pr['a_stop']:
            W = wins[wl]
            denf = mp.tile([128, 4], dt.float32, tag="denf")
            rec = mp.tile([128, 4], dt.float32, tag="rec")
            nc.scalar.activation(out=denf[:], in_=psA[:, HID:HID + 4],
                                 func=mybir.ActivationFunctionType.Copy,
                                 bias=EPS)
            nc.vector.reciprocal(out=rec[:], in_=denf[:])
            for h in range(HEADS):
                dst = (outst[:, wl, h * OUT:(h + 1) * OUT] if last
                       else fdh[:, W, h * OUT:(h + 1) * OUT])
                nc.scalar.activation(
                    out=dst, in_=psA[:, h * OUT:(h + 1) * OUT],
                    func=mybir.ActivationFunctionType.Copy,
                    scale=rec[:, h:h + 1])
    if last:
        LASTN = NPC - 128 * (NWIN - 1)
        w0 = wins[0]
        fullw = len(wins) if wins[-1] < NWIN - 1 else len(wins) - 1
        if fullw:
            nc.sync.dma_start(
                out_d[w0 * 128:(w0 + fullw) * 128, :].rearrange(
                    "(a p) e -> p a e", p=128),
                outst[:, 0:fullw, :])
        if wins[-1] == NWIN - 1:
            nc.sync.dma_start(
                out_d[(NWIN - 1) * 128:NPC, :],
                outst[0:LASTN, len(wins) - 1, :])


# ---------------------------------------------------------------------------


def kernel(**inputs):
    from concourse.bass_utils import run_bass_kernel_spmd

    src = np.asarray(inputs['src'])
    dst = np.asarray(inputs['dst'])
    atom = np.asarray(inputs['atom_feat']).astype(np.float32)
    Ws_eff, Wd_eff, bs_eff, bd_eff, pos_cnt, zero_cnt, T2inv = _fold_weights(
        inputs['W_src'], inputs['b_src'], inputs['W_dst'], inputs['b_dst'],
        inputs['attn'], inputs['bias'])
    for l in range(LAYERS):
        assert np.abs(bs_eff[l]).max() < 1e-12 and np.abs(bd_eff[l]).max() < 1e-12, \
            "nonzero GAT biases not supported by this kernel build"
    assert (zero_cnt == 0).all(), "zero attention weights not supported"

    P = _prep(src, dst)

    win_np = np.zeros((ATOM_DIM + 1, HID), np.float32)
    win_np[:ATOM_DIM] = np.asarray(inputs['W_in'])
    win_np[ATOM_DIM] = np.asarray(inputs['b_in'])
    wsd_np = np.zeros((128, 2 * LAYERS, 512), np.float32)
    for l in range(LAYERS):
        for kc in range(2):
            wsd_np[:, l * 2 + kc, 0:HID] = Ws_eff[l][kc * 128:(kc + 1) * 128]
            wsd_np[:, l * 2 + kc, HID:512] = Wd_eff[l][kc * 128:(kc + 1) * 128]

    nc = _build(P, pos_cnt)

    in_maps = []
    for c in range(NCORES):
        cd = P['cores'][c]
        at = np.zeros((ATOM_DIM + 1, NPC), np.float32)
        at[:ATOM_DIM] = atom[c * NPC + P['nodes_by_r'][c]].T
        at[ATOM_DIM] = 1.0
        in_maps.append({
            'atomT': _bf(at), 'win': _bf(win_np), 'wsd': _bf(wsd_np),
            'fsi': _wrap16(cd['fs_idx']),
            'ohE': cd['ohE'], 'ohA': cd['ohA'],
            'ident': _bf(np.eye(128, dtype=np.float32)),
        })
    res = run_bass_kernel_spmd(nc, in_maps, core_ids=list(range(NCORES)),
                               trace=bool(os.environ.get('KBT_TRACE')))
    kernel._last = res
    full = np.empty((N, HID), np.float64)
    for c in range(NCORES):
        full[c * NPC + P['nodes_by_r'][c]] = res.results[c]['out']
    full = full @ T2inv + np.asarray(inputs['bias'])[LAYERS - 1][None]
    return full.astype(np.float32)


if __name__ == '__main__':
    import jax
    with jax.default_device(jax.devices('cpu')[0]):
        import reference
        inputs = {k: np.asarray(v) for k, v in reference.setup_inputs().items()}
    got = kernel(**inputs)
    print("kernel out:", got.shape, got.dtype, np.abs(got).mean())


# revision 11
# speedup vs baseline: 1.5302x; 1.1822x over previous
"""Trainium2 8-core GATv2 message-passing kernel (nn_AtomGraphEncoder).

Design (v3 — PE-onehot, quarter-split pipelined AllGather):
- Nodes block-sharded 8x12500, degree-balanced permutation into 98 windows
  of 128 nodes per core; edges assigned to dst's core.
- fs table split into 4 quarter tensors (one per 3125-row slice of every
  core); each quarter AllGathered as soon as projection covers it, so the
  collective pipelines behind the projection and the first edge gathers.
- Per layer, per group of 6 windows: dma_gather fs rows (the only
  GPSIMD-heavy op), expand fd per edge-slot via one-hot matmuls (PE),
  alpha-folded prelu (ACT) + head reduces (DVE) for GATv2 logits, exp,
  msg = ex*fs, segment-sum aggregation via transposed one-hot matmuls
  into per-window PSUM (PE) — no dma_scatter_add, no HBM accumulator.
- |a| and the 0.2 lrelu factor folded into W columns on host
  (pos: a, neg: -0.2|a| with alpha=5 prelu), undone on output.
"""
import sys
import os

import numpy as np
import ml_dtypes

sys.path.insert(0, '/opt/trn_rl_repo')

N, E = 100000, 400000
ATOM_DIM, HID, LAYERS, HEADS = 74, 256, 3, 4
OUT = HID // HEADS
NCORES = 8
NPC = N // NCORES            # 12500
NPQ = NPC // 4               # 3125 rows per core per quarter
QTR = NPQ * NCORES           # 25000 rows per quarter table
CLS = 4
NWIN = (NPC + 127) // 128    # 98
G = 6
NGRP = (NWIN + G - 1) // G   # 17
BF = ml_dtypes.bfloat16
EPS = 1e-20
ALPHA = 0.2

# ---------------------------------------------------------------------------
# host prep


def _fold_weights(W_src, b_src, W_dst, b_dst, attn, bias):
    Ts, Tinvs = [], []
    pos_cnt = np.zeros((LAYERS, HEADS), np.int64)
    zero_cnt = np.zeros((LAYERS, HEADS), np.int64)
    for l in range(LAYERS):
        Tl = np.zeros((HID, HID), np.float64)
        Tinv = np.zeros((HID, HID), np.float64)
        for h in range(HEADS):
            a = np.asarray(attn)[l, h].astype(np.float64)
            order = np.concatenate([
                np.where(a > 0)[0], np.where(a == 0)[0], np.where(a < 0)[0]])
            pos_cnt[l, h] = (a > 0).sum()
            zero_cnt[l, h] = (a == 0).sum()
            for j, p in enumerate(order):
                if a[p] > 0:
                    s = a[p]
                elif a[p] == 0:
                    s = 1.0
                else:
                    s = -ALPHA * abs(a[p])
                Tl[h * OUT + p, h * OUT + j] = s
                Tinv[h * OUT + j, h * OUT + p] = 1.0 / s
        Ts.append(Tl)
        Tinvs.append(Tinv)
    Ws_eff, Wd_eff, bs_eff, bd_eff = [], [], [], []
    for l in range(LAYERS):
        Tp = np.eye(HID) if l == 0 else Tinvs[l - 1]
        Ws = np.asarray(W_src)[l].astype(np.float64)
        Wd = np.asarray(W_dst)[l].astype(np.float64)
        bprev = np.zeros(HID) if l == 0 else np.asarray(bias)[l - 1].astype(np.float64)
        Ws_eff.append((Tp @ Ws @ Ts[l]).astype(np.float32))
        Wd_eff.append((Tp @ Wd @ Ts[l]).astype(np.float32))
        bs_eff.append(((np.asarray(b_src)[l] + bprev @ Ws) @ Ts[l]).astype(np.float32))
        bd_eff.append(((np.asarray(b_dst)[l] + bprev @ Wd) @ Ts[l]).astype(np.float32))
    return Ws_eff, Wd_eff, bs_eff, bd_eff, pos_cnt, zero_cnt, Tinvs[-1]


def _balanced_perm(deg):
    caps = np.full(NWIN, 128, np.int64)
    caps[-1] = NPC - 128 * (NWIN - 1)
    order = np.argsort(-deg, kind="stable")
    fill = np.zeros(NWIN, np.int64)
    r_of_node = np.empty(NPC, np.int64)
    seq = np.concatenate([np.arange(NWIN), np.arange(NWIN)[::-1]])
    ptr = 0
    for node in order:
        while fill[seq[ptr % (2 * NWIN)]] >= caps[seq[ptr % (2 * NWIN)]]:
            ptr += 1
        w = seq[ptr % (2 * NWIN)]
        r_of_node[node] = w * 128 + fill[w]
        fill[w] += 1
        ptr += 1
    nodes_by_r = np.empty(NPC, np.int64)
    nodes_by_r[r_of_node] = np.arange(NPC)
    return r_of_node, nodes_by_r


def _prep(src, dst):
    src = np.asarray(src).astype(np.int64)
    dst = np.asarray(dst).astype(np.int64)

    cores_edges = []
    r_of_node_all = []
    nodes_by_r_all = []
    for c in range(NCORES):
        m = (dst >= c * NPC) & (dst < (c + 1) * NPC)
        es, dl = src[m], dst[m] - c * NPC
        deg = np.bincount(dl, minlength=NPC)
        r_of_node, nodes_by_r = _balanced_perm(deg)
        cores_edges.append((es, r_of_node[dl]))
        r_of_node_all.append(r_of_node)
        nodes_by_r_all.append(nodes_by_r)

    # quarter table row for each absolute node id:
    # class = r//NPQ, idx in table = core*NPQ + r%NPQ
    fscls = np.empty(N, np.int64)
    fsidx = np.empty(N, np.int64)
    for c in range(NCORES):
        r = r_of_node_all[c]
        fscls[c * NPC:(c + 1) * NPC] = r // NPQ
        fsidx[c * NPC:(c + 1) * NPC] = c * NPQ + (r % NPQ)

    cnt = np.zeros((NCORES, NWIN, CLS), np.int64)
    core_wk = []
    for c in range(NCORES):
        es, r_d = cores_edges[c]
        w = r_d // 128
        k = fscls[es]
        np.add.at(cnt[c], (w, k), 1)
        core_wk.append((w, k))
    seg = cnt.max(axis=0)

    run_base = np.zeros((NWIN, CLS), np.int64)
    pieces = [[] for _ in range(NGRP)]
    group_cols = []
    off = 0
    for g in range(NGRP):
        wins = list(range(g * G, min(NWIN, g * G + G)))
        g0 = off
        for k in range(CLS):
            cstart = off
            for w in wins:
                run_base[w, k] = off
                off += seg[w, k]
            off += (-(off - cstart)) % 128
            if off > cstart:
                pieces[g].append((k, cstart, off - cstart))
        group_cols.append((g0 // 128, off // 128))
    S = off
    T_all = S // 128

    pairs = [[] for _ in range(NGRP)]
    PM = np.full((T_all, G), -1, np.int64)
    pi = 0
    for g in range(NGRP):
        wins = list(range(g * G, min(NWIN, g * G + G)))
        c0, c1 = group_cols[g]
        plist = []
        for col in range(c0, c1):
            a, b = col * 128, (col + 1) * 128
            for wl, w in enumerate(wins):
                for k in range(CLS):
                    lo = max(run_base[w, k], a)
                    hi = min(run_base[w, k] + seg[w, k], b)
                    if lo < hi:
                        plist.append((col - c0, wl))
                        break
        seen = set()
        plist2 = []
        for p in plist:
            if p not in seen:
                seen.add(p)
                plist2.append(p)
        wl_first, wl_last, col_first, col_last = {}, {}, {}, {}
        for i, (cl, wl) in enumerate(plist2):
            wl_first.setdefault(wl, i)
            wl_last[wl] = i
            col_first.setdefault(cl, i)
            col_last[cl] = i
        for i, (cl, wl) in enumerate(plist2):
            pairs[g].append(dict(
                c=cl, w=wl, pi=pi,
                e_start=(col_first[cl] == i), e_stop=(col_last[cl] == i),
                a_start=(wl_first[wl] == i), a_stop=(wl_last[wl] == i)))
            PM[c0 + cl, wl] = pi
            pi += 1
        for w in range(len(wins)):
            assert w in wl_first, f"window {g * G + w} has no pairs"
    P_total = pi
    Pmax = max(len(p) for p in pairs)
    Cmax = max(c1 - c0 for c0, c1 in group_cols)

    cores = []
    for c in range(NCORES):
        es, r_d = cores_edges[c]
        w, k = core_wk[c]
        order = np.lexsort((r_d, k, w))
        es_s, rd_s, w_s, k_s = es[order], r_d[order], w[order], k[order]
        key = w_s * CLS + k_s
        uniq, starts = np.unique(key, return_index=True)
        rank = np.arange(len(key)) - np.repeat(
            starts, np.diff(np.concatenate([starts, [len(key)]])))
        slot = run_base[w_s, k_s] + rank
        fs_idx = np.zeros(S, np.int64)
        fs_idx[slot] = fsidx[es_s]
        g_s = w_s // G
        wl_s = w_s - g_s * G
        pi_e = PM[slot // 128, wl_s]
        assert (pi_e >= 0).all()
        ohA = np.zeros((128, P_total, 128), BF)
        ohE = np.zeros((128, P_total, 128), BF)
        ohA[slot % 128, pi_e, rd_s % 128] = 1.0
        ohE[rd_s % 128, pi_e, slot % 128] = 1.0
        cores.append(dict(fs_idx=fs_idx, ohA=ohA, ohE=ohE))

    return dict(seg=seg, run_base=run_base, pieces=pieces, pairs=pairs,
                group_cols=group_cols, S=S, T_all=T_all, P_total=P_total,
                Pmax=Pmax, Cmax=Cmax, cores=cores,
                nodes_by_r=nodes_by_r_all)


def _bf(x):
    return np.asarray(x).astype(BF)


def _wrap16(idx):
    w = np.ascontiguousarray(np.asarray(idx).reshape(-1, 16).T).astype(np.int16)
    return np.tile(w, (8, 1))


# ---------------------------------------------------------------------------
# bass build


def _build(P, pos_cnt):
    import concourse.bass as bass
    import concourse.tile as tile
    from concourse import bacc, mybir, library_config

    S = P['S']
    Pmax, Cmax, P_total = P['Pmax'], P['Cmax'], P['P_total']

    nc = bacc.Bacc("TRN2", target_bir_lowering=False, debug=False,
                   num_devices=NCORES)
    dt = mybir.dt
    atomT_d = nc.dram_tensor("atomT", [ATOM_DIM + 1, NPC], dt.bfloat16,
                             kind="ExternalInput")
    win_d = nc.dram_tensor("win", [ATOM_DIM + 1, HID], dt.bfloat16,
                           kind="ExternalInput")
    wsd_d = nc.dram_tensor("wsd", [128, 2 * LAYERS, 512], dt.bfloat16,
                           kind="ExternalInput")
    fsi_d = nc.dram_tensor("fsi", [128, S // 16], dt.int16, kind="ExternalInput")
    ohE_d = nc.dram_tensor("ohE", [128, P_total, 128], dt.bfloat16,
                           kind="ExternalInput")
    ohA_d = nc.dram_tensor("ohA", [128, P_total, 128], dt.bfloat16,
                           kind="ExternalInput")
    ident_d = nc.dram_tensor("ident", [128, 128], dt.bfloat16,
                             kind="ExternalInput")
    out_d = nc.dram_tensor("out", [NPC, HID], dt.float32, kind="ExternalOutput")

    fs_bounce = nc.dram_tensor("fs_bounce", [NPC, HID], dt.bfloat16)
    # double-buffered across layers: AG for layer l+1 streams during edge(l)
    fs_q = [[nc.dram_tensor(f"fs_q{b}_{k}", [QTR, HID], dt.bfloat16,
                            addr_space="Shared") for k in range(CLS)]
            for b in range(2)]

    LASTN = NPC - 128 * (NWIN - 1)   # 84
    AFT = mybir.ActivationFunctionType

    with tile.TileContext(nc) as tc:
        nc.gpsimd.load_library(library_config.mlp)
        with tc.tile_pool(name="persist", bufs=1) as pp, \
             tc.tile_pool(name="atp", bufs=2) as ap_, \
             tc.tile_pool(name="htp", bufs=3) as hp, \
             tc.tile_pool(name="stage", bufs=2) as sp, \
             tc.tile_pool(name="ohp", bufs=2) as op_, \
             tc.tile_pool(name="fsgp", bufs=2) as wp, \
             tc.tile_pool(name="upp", bufs=2) as up_, \
             tc.tile_pool(name="payp", bufs=2) as yp, \
             tc.tile_pool(name="smallp", bufs=2) as mp, \
             tc.tile_pool(name="psE", bufs=2, space="PSUM") as psE, \
             tc.tile_pool(name="aggp", bufs=1, space="PSUM") as aggp:

            fsi = pp.tile([128, S // 16], dt.int16, tag="fsi")
            wsd = pp.tile([128, 2 * LAYERS, 512], dt.bfloat16, tag="wsd")
            win = pp.tile([ATOM_DIM + 1, HID], dt.bfloat16, tag="win")
            fdh = pp.tile([128, NWIN, HID], dt.bfloat16, tag="fdh")
            ident = pp.tile([128, 128], dt.bfloat16, tag="ident")
            nc.sync.dma_start(ident[:], ident_d[:])
            nc.sync.dma_start(fsi[:], fsi_d[:])
            nc.sync.dma_start(wsd[:], wsd_d[:])
            nc.sync.dma_start(win[:], win_d[:])

            def fire_ags(lo_row, hi_row, lset):
                """AllGather any fs quarter fully covered in [lo_row, hi_row)."""
                for q in range(CLS):
                    if lo_row < (q + 1) * NPQ <= hi_row:
                        nc.gpsimd.collective_compute(
                            "AllGather", mybir.AluOpType.bypass,
                            replica_groups=[list(range(NCORES))],
                            ins=[fs_bounce[q * NPQ:(q + 1) * NPQ, :].opt()],
                            outs=[fs_q[lset][q][:].opt()])

            def proj_block(l, a, fs_sb, j, ptag0, ptag1):
                """Project window a of layer l: fdh h -> (fs staging, fdh fd)."""
                nt = 128 if a < NWIN - 1 else LASTN
                hTst = hp.tile([128, 2, 128], dt.bfloat16, tag="hTst",
                               name="hTst")
                for cch in range(2):
                    tg = ptag0 if cch == 0 else ptag1
                    pt = aggp.tile([128, 128], dt.bfloat16,
                                   tag=f"agg{tg}", name=f"tp{tg}")
                    nc.tensor.transpose(
                        pt[:], fdh[:, a, cch * 128:(cch + 1) * 128], ident[:])
                    if cch == 0:
                        nc.scalar.activation(out=hTst[:, cch, :], in_=pt[:],
                                             func=AFT.Copy)
                    else:
                        nc.vector.tensor_copy(out=hTst[:, cch, :], in_=pt[:])
                ps = psE.tile([128, 512], dt.float32, tag="pse", name="ps")
                for kc in range(2):
                    nc.tensor.matmul(ps[0:nt, :], hTst[:, kc, 0:nt],
                                     wsd[:, l * 2 + kc, :],
                                     start=(kc == 0), stop=(kc == 1))
                nc.scalar.activation(out=fs_sb[0:nt, j, :], in_=ps[0:nt, 0:HID],
                                     func=AFT.Copy)
                nc.vector.tensor_copy(out=fdh[0:nt, a, :], in_=ps[0:nt, HID:512])

            def flush_fs(a0, a1, fs_sb):
                """DMA staged fs windows [a0, a1) to fs_bounce."""
                nw = a1 - a0
                fullw = nw if a1 - 1 < NWIN - 1 else nw - 1
                if fullw:
                    nc.sync.dma_start(
                        fs_bounce[a0 * 128:(a0 + fullw) * 128, :].rearrange(
                            "(a p) e -> p a e", p=128),
                        fs_sb[:, 0:fullw, :])
                if fullw < nw:
                    nc.sync.dma_start(
                        fs_bounce[(NWIN - 1) * 128:NPC, :],
                        fs_sb[0:LASTN, nw - 1, :])

            # ---- input projection: fdh = h0 (node-major, permuted order)
            nc.vector.memset(fdh[:, NWIN - 1, :], 0.0)
            for a in range(NWIN):
                nt = 128 if a < NWIN - 1 else LASTN
                at = ap_.tile([ATOM_DIM + 1, 128], dt.bfloat16, tag="at")
                nc.sync.dma_start(at[:, 0:nt], atomT_d[:, a * 128:a * 128 + nt])
                ps = psE.tile([128, 512], dt.float32, tag="pse", name="ps")
                nc.tensor.matmul(ps[0:nt, 0:HID], at[:, 0:nt], win[:],
                                 start=True, stop=True)
                nc.scalar.activation(out=fdh[0:nt, a, :], in_=ps[0:nt, 0:HID],
                                     func=AFT.Copy)

            # ---- standalone projection for layer 0 (edge(-1) doesn't exist)
            for a in range(NWIN):
                j = a % 8
                if j == 0:
                    fs_sb = sp.tile([128, 8, HID], dt.bfloat16, tag="fs_sb",
                                    name="fs_sb")
                proj_block(0, a, fs_sb, j, a & 1, 2 + (a & 1))
                if j == 7 or a == NWIN - 1:
                    flush_fs(a - j, a + 1, fs_sb)
                    fire_ags((a - j) * 128, min(NPC, (a + 1) * 128), 0)

            # ---- per-layer edge phase; proj(l+1) + its AGs hide inside
            for l in range(LAYERS):
                last = l == LAYERS - 1
                holdover = None
                for g in range(NGRP):
                    c0, c1 = P['group_cols'][g]
                    Cg = c1 - c0
                    wins = list(range(g * G, min(NWIN, g * G + G)))
                    pg = P['pairs'][g]
                    Pg = len(pg)
                    pbase = pg[0]['pi']

                    ohEt = op_.tile([128, Pmax, 128], dt.bfloat16, tag="ohE")
                    ohAt = op_.tile([128, Pmax, 128], dt.bfloat16, tag="ohA")
                    nc.sync.dma_start(ohEt[:, 0:Pg, :],
                                      ohE_d[:, pbase:pbase + Pg, :])
                    nc.sync.dma_start(ohAt[:, 0:Pg, :],
                                      ohA_d[:, pbase:pbase + Pg, :])
                    fsg = wp.tile([128, Cmax, HID], dt.bfloat16, tag="fsg")
                    for (k, soff, n) in P['pieces'][g]:
                        nc.gpsimd.dma_gather(
                            fsg[:, soff // 128 - c0:(soff + n) // 128 - c0, :],
                            fs_q[l % 2][k][:],
                            fsi[:, soff // 16:(soff + n) // 16], n, n, HID)

                    # expand fd per slot-col (2 cols per PSUM tile) + u = fs+fd
                    upre = up_.tile([128, Cmax, HID], dt.bfloat16, tag="upre",
                                    bufs=1)
                    percol = {}
                    for i, pr in enumerate(pg):
                        percol.setdefault(pr['c'], []).append(i)
                    for cl0 in range(0, Cg, 2):
                        ncol = min(2, Cg - cl0)
                        ps = psE.tile([128, 512], dt.float32, tag="pse",
                                      name="ps")
                        for dc in range(ncol):
                            cl = cl0 + dc
                            idxs = percol[cl]
                            for ii, i in enumerate(idxs):
                                pr = pg[i]
                                nc.tensor.matmul(
                                    ps[:, dc * HID:(dc + 1) * HID],
                                    ohEt[:, i, :], fdh[:, wins[pr['w']], :],
                                    start=(ii == 0), stop=(ii == len(idxs) - 1))
                        nc.vector.tensor_tensor(
                            out=upre[:, cl0:cl0 + ncol, :],
                            in0=ps[:, 0:ncol * HID].rearrange(
                                "p (t e) -> p t e", t=ncol),
                            in1=fsg[:, cl0:cl0 + ncol, :],
                            op=mybir.AluOpType.add)

                    # alpha-folded prelu + per-head logits
                    for h in range(HEADS):
                        kp = int(pos_cnt[l, h])
                        if kp:
                            nc.scalar.activation(
                                out=upre[:, 0:Cg, h * OUT:h * OUT + kp],
                                in_=upre[:, 0:Cg, h * OUT:h * OUT + kp],
                                func=AFT.Prelu, alpha=ALPHA)
                        if kp < OUT:
                            nc.scalar.activation(
                                out=upre[:, 0:Cg, h * OUT + kp:(h + 1) * OUT],
                                in_=upre[:, 0:Cg, h * OUT + kp:(h + 1) * OUT],
                                func=AFT.Prelu, alpha=1.0 / ALPHA)
                    lg = mp.tile([128, Cmax, 4], dt.float32, tag="lg")
                    for h in range(HEADS):
                        nc.vector.tensor_reduce(
                            out=lg[:, 0:Cg, h],
                            in_=upre[:, 0:Cg, h * OUT:(h + 1) * OUT],
                            axis=mybir.AxisListType.X, op=mybir.AluOpType.add)
                    pay = yp.tile([128, Cmax, HID + 4], dt.bfloat16, tag="pay")
                    nc.scalar.activation(out=pay[:, 0:Cg, HID:HID + 4],
                                         in_=lg[:, 0:Cg, :], func=AFT.Exp)
                    nc.vector.tensor_tensor(
                        out=pay[:, 0:Cg, 0:HID].rearrange(
                            "p t (h d) -> p t h d", h=HEADS),
                        in0=fsg[:, 0:Cg, :].rearrange(
                            "p t (h d) -> p t h d", h=HEADS),
                        in1=pay[:, 0:Cg, HID:HID + 4].unsqueeze(3).broadcast_to(
                            [128, Cg, HEADS, OUT]),
                        op=mybir.AluOpType.mult)

                    if holdover is not None:
                        _agg_norm(nc, mybir, aggp, mp, sp, fdh, out_d,
                                  holdover, last, proj_block, flush_fs,
                                  fire_ags, l)
                    holdover = (g, wins, pg, ohAt, pay)
                if holdover is not None:
                    _agg_norm(nc, mybir, aggp, mp, sp, fdh, out_d,
                              holdover, last, proj_block, flush_fs,
                              fire_ags, l)
    nc.compile()
    return nc


def _agg_norm(nc, mybir, aggp, mp, sp, fdh, out_d, holdover, last,
              proj_block, flush_fs, fire_ags, l):
    dt = mybir.dt
    AFT = mybir.ActivationFunctionType
    g, wins, pg, ohAt, pay = holdover
    cur = {}
    outst = None
    fs_sb = None
    if last:
        outst = sp.tile([128, G, HID], dt.float32, tag="outst", bufs=1)
    else:
        fs_sb = sp.tile([128, G, HID], dt.bfloat16, tag="fs_sb", name="fs_sb")
    for i, pr in enumerate(pg):
        wl = pr['w']
        if pr['a_start']:
            cur[wl] = aggp.tile([128, HID + 4], dt.float32, tag=f"agg{wl}",
                                name=f"aggps{wl}")
        psA = cur[wl]
        nc.tensor.matmul(psA[:, 0:HID + 4], ohAt[:, i, :],
                         pay[:, pr['c'], 0:HID + 4],
                         start=pr['a_start'], stop=pr['a_stop'])
        if pr['a_stop']:
            W = wins[wl]
            denf = mp.tile([128, 4], dt.float32, tag="denf")
            rec = mp.tile([128, 4], dt.float32, tag="rec")
            nc.scalar.activation(out=denf[:], in_=psA[:, HID:HID + 4],
                                 func=AFT.Copy, bias=EPS)
            nc.vector.reciprocal(out=rec[:], in_=denf[:])
            for h in range(4):
                dst = (outst[:, wl, h * OUT:(h + 1) * OUT] if last
                       else fdh[:, W, h * OUT:(h + 1) * OUT])
                nc.scalar.activation(
                    out=dst, in_=psA[:, h * OUT:(h + 1) * OUT],
                    func=AFT.Copy, scale=rec[:, h:h + 1])
            if not last:
                # project window W for layer l+1 (hides proj + AG in edge)
                proj_block(l + 1, W, fs_sb, wl, wl, wl)
    w0, w1 = wins[0], wins[-1] + 1
    if last:
        LASTN = NPC - 128 * (NWIN - 1)
        fullw = len(wins) if wins[-1] < NWIN - 1 else len(wins) - 1
        if fullw:
            nc.sync.dma_start(
                out_d[w0 * 128:(w0 + fullw) * 128, :].rearrange(
                    "(a p) e -> p a e", p=128),
                outst[:, 0:fullw, :])
        if wins[-1] == NWIN - 1:
            nc.sync.dma_start(
                out_d[(NWIN - 1) * 128:NPC, :],
                outst[0:LASTN, len(wins) - 1, :])
    else:
        flush_fs(w0, w1, fs_sb)
        fire_ags(w0 * 128, min(NPC, w1 * 128), (l + 1) % 2)


# ---------------------------------------------------------------------------


def kernel(**inputs):
    from concourse.bass_utils import run_bass_kernel_spmd

    src = np.asarray(inputs['src'])
    dst = np.asarray(inputs['dst'])
    atom = np.asarray(inputs['atom_feat']).astype(np.float32)
    Ws_eff, Wd_eff, bs_eff, bd_eff, pos_cnt, zero_cnt, T2inv = _fold_weights(
        inputs['W_src'], inputs['b_src'], inputs['W_dst'], inputs['b_dst'],
        inputs['attn'], inputs['bias'])
    for l in range(LAYERS):
        assert np.abs(bs_eff[l]).max() < 1e-12 and np.abs(bd_eff[l]).max() < 1e-12, \
            "nonzero GAT biases not supported by this kernel build"
    assert (zero_cnt == 0).all(), "zero attention weights not supported"

    P = _prep(src, dst)

    win_np = np.zeros((ATOM_DIM + 1, HID), np.float32)
    win_np[:ATOM_DIM] = np.asarray(inputs['W_in'])
    win_np[ATOM_DIM] = np.asarray(inputs['b_in'])
    wsd_np = np.zeros((128, 2 * LAYERS, 512), np.float32)
    for l in range(LAYERS):
        for kc in range(2):
            wsd_np[:, l * 2 + kc, 0:HID] = Ws_eff[l][kc * 128:(kc + 1) * 128]
            wsd_np[:, l * 2 + kc, HID:512] = Wd_eff[l][kc * 128:(kc + 1) * 128]

    nc = _build(P, pos_cnt)

    in_maps = []
    for c in range(NCORES):
        cd = P['cores'][c]
        at = np.zeros((ATOM_DIM + 1, NPC), np.float32)
        at[:ATOM_DIM] = atom[c * NPC + P['nodes_by_r'][c]].T
        at[ATOM_DIM] = 1.0
        in_maps.append({
            'atomT': _bf(at), 'win': _bf(win_np), 'wsd': _bf(wsd_np),
            'fsi': _wrap16(cd['fs_idx']),
            'ohE': cd['ohE'], 'ohA': cd['ohA'],
            'ident': _bf(np.eye(128, dtype=np.float32)),
        })
    res = run_bass_kernel_spmd(nc, in_maps, core_ids=list(range(NCORES)),
                               trace=bool(os.environ.get('KBT_TRACE')))
    kernel._last = res
    full = np.empty((N, HID), np.float64)
    for c in range(NCORES):
        full[c * NPC + P['nodes_by_r'][c]] = res.results[c]['out']
    full = full @ T2inv + np.asarray(inputs['bias'])[LAYERS - 1][None]
    return full.astype(np.float32)


if __name__ == '__main__':
    import jax
    with jax.default_device(jax.devices('cpu')[0]):
        import reference
        inputs = {k: np.asarray(v) for k, v in reference.setup_inputs().items()}
    got = kernel(**inputs)
    print("kernel out:", got.shape, got.dtype, np.abs(got).mean())


# revision 12
# speedup vs baseline: 1.5542x; 1.0157x over previous
"""Trainium2 8-core GATv2 message-passing kernel (nn_AtomGraphEncoder).

Design (v3 — PE-onehot, quarter-split pipelined AllGather):
- Nodes block-sharded 8x12500, degree-balanced permutation into 98 windows
  of 128 nodes per core; edges assigned to dst's core.
- fs table split into 4 quarter tensors (one per 3125-row slice of every
  core); each quarter AllGathered as soon as projection covers it, so the
  collective pipelines behind the projection and the first edge gathers.
- Per layer, per group of 6 windows: dma_gather fs rows (the only
  GPSIMD-heavy op), expand fd per edge-slot via one-hot matmuls (PE),
  alpha-folded prelu (ACT) + head reduces (DVE) for GATv2 logits, exp,
  msg = ex*fs, segment-sum aggregation via transposed one-hot matmuls
  into per-window PSUM (PE) — no dma_scatter_add, no HBM accumulator.
- |a| and the 0.2 lrelu factor folded into W columns on host
  (pos: a, neg: -0.2|a| with alpha=5 prelu), undone on output.
"""
import sys
import os

import numpy as np
import ml_dtypes

sys.path.insert(0, '/opt/trn_rl_repo')

N, E = 100000, 400000
ATOM_DIM, HID, LAYERS, HEADS = 74, 256, 3, 4
OUT = HID // HEADS
NCORES = 8
NPC = N // NCORES            # 12500
NPQ = NPC // 4               # 3125 rows per core per quarter
QTR = NPQ * NCORES           # 25000 rows per quarter table
CLS = 4
NWIN = (NPC + 127) // 128    # 98
G = 6
NGRP = (NWIN + G - 1) // G   # 17
BF = ml_dtypes.bfloat16
EPS = 1e-20
ALPHA = 0.2

# ---------------------------------------------------------------------------
# host prep


def _fold_weights(W_src, b_src, W_dst, b_dst, attn, bias):
    Ts, Tinvs = [], []
    pos_cnt = np.zeros((LAYERS, HEADS), np.int64)
    zero_cnt = np.zeros((LAYERS, HEADS), np.int64)
    for l in range(LAYERS):
        Tl = np.zeros((HID, HID), np.float64)
        Tinv = np.zeros((HID, HID), np.float64)
        for h in range(HEADS):
            a = np.asarray(attn)[l, h].astype(np.float64)
            order = np.concatenate([
                np.where(a > 0)[0], np.where(a == 0)[0], np.where(a < 0)[0]])
            pos_cnt[l, h] = (a > 0).sum()
            zero_cnt[l, h] = (a == 0).sum()
            for j, p in enumerate(order):
                if a[p] > 0:
                    s = a[p]
                elif a[p] == 0:
                    s = 1.0
                else:
                    s = -ALPHA * abs(a[p])
                Tl[h * OUT + p, h * OUT + j] = s
                Tinv[h * OUT + j, h * OUT + p] = 1.0 / s
        Ts.append(Tl)
        Tinvs.append(Tinv)
    Ws_eff, Wd_eff, bs_eff, bd_eff = [], [], [], []
    for l in range(LAYERS):
        Tp = np.eye(HID) if l == 0 else Tinvs[l - 1]
        Ws = np.asarray(W_src)[l].astype(np.float64)
        Wd = np.asarray(W_dst)[l].astype(np.float64)
        bprev = np.zeros(HID) if l == 0 else np.asarray(bias)[l - 1].astype(np.float64)
        Ws_eff.append((Tp @ Ws @ Ts[l]).astype(np.float32))
        Wd_eff.append((Tp @ Wd @ Ts[l]).astype(np.float32))
        bs_eff.append(((np.asarray(b_src)[l] + bprev @ Ws) @ Ts[l]).astype(np.float32))
        bd_eff.append(((np.asarray(b_dst)[l] + bprev @ Wd) @ Ts[l]).astype(np.float32))
    return Ws_eff, Wd_eff, bs_eff, bd_eff, pos_cnt, zero_cnt, Tinvs[-1]


def _balanced_perm(deg):
    caps = np.full(NWIN, 128, np.int64)
    caps[-1] = NPC - 128 * (NWIN - 1)
    order = np.argsort(-deg, kind="stable")
    fill = np.zeros(NWIN, np.int64)
    r_of_node = np.empty(NPC, np.int64)
    seq = np.concatenate([np.arange(NWIN), np.arange(NWIN)[::-1]])
    ptr = 0
    for node in order:
        while fill[seq[ptr % (2 * NWIN)]] >= caps[seq[ptr % (2 * NWIN)]]:
            ptr += 1
        w = seq[ptr % (2 * NWIN)]
        r_of_node[node] = w * 128 + fill[w]
        fill[w] += 1
        ptr += 1
    nodes_by_r = np.empty(NPC, np.int64)
    nodes_by_r[r_of_node] = np.arange(NPC)
    return r_of_node, nodes_by_r


def _prep(src, dst):
    src = np.asarray(src).astype(np.int64)
    dst = np.asarray(dst).astype(np.int64)

    cores_edges = []
    r_of_node_all = []
    nodes_by_r_all = []
    for c in range(NCORES):
        m = (dst >= c * NPC) & (dst < (c + 1) * NPC)
        es, dl = src[m], dst[m] - c * NPC
        deg = np.bincount(dl, minlength=NPC)
        r_of_node, nodes_by_r = _balanced_perm(deg)
        cores_edges.append((es, r_of_node[dl]))
        r_of_node_all.append(r_of_node)
        nodes_by_r_all.append(nodes_by_r)

    # quarter table row for each absolute node id:
    # class = r//NPQ, idx in table = core*NPQ + r%NPQ
    fscls = np.empty(N, np.int64)
    fsidx = np.empty(N, np.int64)
    for c in range(NCORES):
        r = r_of_node_all[c]
        fscls[c * NPC:(c + 1) * NPC] = r // NPQ
        fsidx[c * NPC:(c + 1) * NPC] = c * NPQ + (r % NPQ)

    cnt = np.zeros((NCORES, NWIN, CLS), np.int64)
    core_wk = []
    for c in range(NCORES):
        es, r_d = cores_edges[c]
        w = r_d // 128
        k = fscls[es]
        np.add.at(cnt[c], (w, k), 1)
        core_wk.append((w, k))
    seg = cnt.max(axis=0)

    run_base = np.zeros((NWIN, CLS), np.int64)
    pieces = [[] for _ in range(NGRP)]
    group_cols = []
    off = 0
    for g in range(NGRP):
        wins = list(range(g * G, min(NWIN, g * G + G)))
        g0 = off
        for k in range(CLS):
            cstart = off
            for w in wins:
                run_base[w, k] = off
                off += seg[w, k]
            off += (-(off - cstart)) % 128
            if off > cstart:
                pieces[g].append((k, cstart, off - cstart))
        group_cols.append((g0 // 128, off // 128))
    S = off
    T_all = S // 128

    pairs = [[] for _ in range(NGRP)]
    PM = np.full((T_all, G), -1, np.int64)
    pi = 0
    for g in range(NGRP):
        wins = list(range(g * G, min(NWIN, g * G + G)))
        c0, c1 = group_cols[g]
        plist = []
        for col in range(c0, c1):
            a, b = col * 128, (col + 1) * 128
            for wl, w in enumerate(wins):
                for k in range(CLS):
                    lo = max(run_base[w, k], a)
                    hi = min(run_base[w, k] + seg[w, k], b)
                    if lo < hi:
                        plist.append((col - c0, wl))
                        break
        seen = set()
        plist2 = []
        for p in plist:
            if p not in seen:
                seen.add(p)
                plist2.append(p)
        wl_first, wl_last, col_first, col_last = {}, {}, {}, {}
        for i, (cl, wl) in enumerate(plist2):
            wl_first.setdefault(wl, i)
            wl_last[wl] = i
            col_first.setdefault(cl, i)
            col_last[cl] = i
        for i, (cl, wl) in enumerate(plist2):
            pairs[g].append(dict(
                c=cl, w=wl, pi=pi,
                e_start=(col_first[cl] == i), e_stop=(col_last[cl] == i),
                a_start=(wl_first[wl] == i), a_stop=(wl_last[wl] == i)))
            PM[c0 + cl, wl] = pi
            pi += 1
        for w in range(len(wins)):
            assert w in wl_first, f"window {g * G + w} has no pairs"
    P_total = pi
    Pmax = max(len(p) for p in pairs)
    Cmax = max(c1 - c0 for c0, c1 in group_cols)

    cores = []
    for c in range(NCORES):
        es, r_d = cores_edges[c]
        w, k = core_wk[c]
        order = np.lexsort((r_d, k, w))
        es_s, rd_s, w_s, k_s = es[order], r_d[order], w[order], k[order]
        key = w_s * CLS + k_s
        uniq, starts = np.unique(key, return_index=True)
        rank = np.arange(len(key)) - np.repeat(
            starts, np.diff(np.concatenate([starts, [len(key)]])))
        slot = run_base[w_s, k_s] + rank
        fs_idx = np.zeros(S, np.int64)
        fs_idx[slot] = fsidx[es_s]
        g_s = w_s // G
        wl_s = w_s - g_s * G
        pi_e = PM[slot // 128, wl_s]
        assert (pi_e >= 0).all()
        ohA = np.zeros((128, P_total, 128), BF)
        ohE = np.zeros((128, P_total, 128), BF)
        ohA[slot % 128, pi_e, rd_s % 128] = 1.0
        ohE[rd_s % 128, pi_e, slot % 128] = 1.0
        cores.append(dict(fs_idx=fs_idx, ohA=ohA, ohE=ohE))

    return dict(seg=seg, run_base=run_base, pieces=pieces, pairs=pairs,
                group_cols=group_cols, S=S, T_all=T_all, P_total=P_total,
                Pmax=Pmax, Cmax=Cmax, cores=cores,
                nodes_by_r=nodes_by_r_all)


def _bf(x):
    return np.asarray(x).astype(BF)


def _wrap16(idx):
    w = np.ascontiguousarray(np.asarray(idx).reshape(-1, 16).T).astype(np.int16)
    return np.tile(w, (8, 1))


# ---------------------------------------------------------------------------
# bass build


def _build(P, pos_cnt):
    import concourse.bass as bass
    import concourse.tile as tile
    from concourse import bacc, mybir, library_config

    S = P['S']
    Pmax, Cmax, P_total = P['Pmax'], P['Cmax'], P['P_total']

    nc = bacc.Bacc("TRN2", target_bir_lowering=False, debug=False,
                   num_devices=NCORES)
    dt = mybir.dt
    atomT_d = nc.dram_tensor("atomT", [ATOM_DIM + 1, NPC], dt.bfloat16,
                             kind="ExternalInput")
    win_d = nc.dram_tensor("win", [ATOM_DIM + 1, HID], dt.bfloat16,
                           kind="ExternalInput")
    wsd_d = nc.dram_tensor("wsd", [128, 2 * LAYERS, 512], dt.bfloat16,
                           kind="ExternalInput")
    fsi_d = nc.dram_tensor("fsi", [128, S // 16], dt.int16, kind="ExternalInput")
    ohE_d = nc.dram_tensor("ohE", [128, P_total, 128], dt.bfloat16,
                           kind="ExternalInput")
    ohA_d = nc.dram_tensor("ohA", [128, P_total, 128], dt.bfloat16,
                           kind="ExternalInput")
    ident_d = nc.dram_tensor("ident", [128, 128], dt.bfloat16,
                             kind="ExternalInput")
    out_d = nc.dram_tensor("out", [NPC, HID], dt.float32, kind="ExternalOutput")

    fs_bounce = nc.dram_tensor("fs_bounce", [NPC, HID], dt.bfloat16)
    # double-buffered across layers: AG for layer l+1 streams during edge(l)
    fs_q = [[nc.dram_tensor(f"fs_q{b}_{k}", [QTR, HID], dt.bfloat16,
                            addr_space="Shared") for k in range(CLS)]
            for b in range(2)]

    LASTN = NPC - 128 * (NWIN - 1)   # 84
    AFT = mybir.ActivationFunctionType

    with tile.TileContext(nc) as tc:
        nc.gpsimd.load_library(library_config.mlp)
        with tc.tile_pool(name="persist", bufs=1) as pp, \
             tc.tile_pool(name="atp", bufs=2) as ap_, \
             tc.tile_pool(name="htp", bufs=3) as hp, \
             tc.tile_pool(name="stage", bufs=2) as sp, \
             tc.tile_pool(name="ohp", bufs=2) as op_, \
             tc.tile_pool(name="fsgp", bufs=2) as wp, \
             tc.tile_pool(name="upp", bufs=2) as up_, \
             tc.tile_pool(name="payp", bufs=2) as yp, \
             tc.tile_pool(name="smallp", bufs=2) as mp, \
             tc.tile_pool(name="psE", bufs=2, space="PSUM") as psE, \
             tc.tile_pool(name="aggp", bufs=1, space="PSUM") as aggp:

            fsi = pp.tile([128, S // 16], dt.int16, tag="fsi")
            wsd = pp.tile([128, 2 * LAYERS, 512], dt.bfloat16, tag="wsd")
            win = pp.tile([ATOM_DIM + 1, HID], dt.bfloat16, tag="win")
            fdh = pp.tile([128, NWIN, HID], dt.bfloat16, tag="fdh")
            ident = pp.tile([128, 128], dt.bfloat16, tag="ident")
            nc.sync.dma_start(ident[:], ident_d[:])
            nc.sync.dma_start(fsi[:], fsi_d[:])
            nc.sync.dma_start(wsd[:], wsd_d[:])
            nc.sync.dma_start(win[:], win_d[:])

            def fire_ags(lo_row, hi_row, lset):
                """AllGather any fs quarter fully covered in [lo_row, hi_row)."""
                for q in range(CLS):
                    if lo_row < (q + 1) * NPQ <= hi_row:
                        nc.gpsimd.collective_compute(
                            "AllGather", mybir.AluOpType.bypass,
                            replica_groups=[list(range(NCORES))],
                            ins=[fs_bounce[q * NPQ:(q + 1) * NPQ, :].opt()],
                            outs=[fs_q[lset][q][:].opt()])

            def proj_block(l, a, fs_sb, j, ptag0, ptag1):
                """Project window a of layer l: fdh h -> (fs staging, fdh fd)."""
                nt = 128 if a < NWIN - 1 else LASTN
                hTst = hp.tile([128, 2, 128], dt.bfloat16, tag="hTst",
                               name="hTst")
                for cch in range(2):
                    tg = ptag0 if cch == 0 else ptag1
                    pt = aggp.tile([128, 128], dt.bfloat16,
                                   tag=f"agg{tg}", name=f"tp{tg}")
                    nc.tensor.transpose(
                        pt[:], fdh[:, a, cch * 128:(cch + 1) * 128], ident[:])
                    if cch == 0:
                        nc.scalar.activation(out=hTst[:, cch, :], in_=pt[:],
                                             func=AFT.Copy)
                    else:
                        nc.vector.tensor_copy(out=hTst[:, cch, :], in_=pt[:])
                ps = psE.tile([128, 512], dt.float32, tag="pse", name="ps")
                for kc in range(2):
                    nc.tensor.matmul(ps[0:nt, :], hTst[:, kc, 0:nt],
                                     wsd[:, l * 2 + kc, :],
                                     start=(kc == 0), stop=(kc == 1))
                nc.scalar.activation(out=fs_sb[0:nt, j, :], in_=ps[0:nt, 0:HID],
                                     func=AFT.Copy)
                nc.vector.tensor_copy(out=fdh[0:nt, a, :], in_=ps[0:nt, HID:512])

            def flush_fs(a0, a1, fs_sb):
                """DMA staged fs windows [a0, a1) to fs_bounce."""
                nw = a1 - a0
                fullw = nw if a1 - 1 < NWIN - 1 else nw - 1
                if fullw:
                    nc.sync.dma_start(
                        fs_bounce[a0 * 128:(a0 + fullw) * 128, :].rearrange(
                            "(a p) e -> p a e", p=128),
                        fs_sb[:, 0:fullw, :])
                if fullw < nw:
                    nc.sync.dma_start(
                        fs_bounce[(NWIN - 1) * 128:NPC, :],
                        fs_sb[0:LASTN, nw - 1, :])

            # ---- input projection fused with layer-0 projection per block
            nc.vector.memset(fdh[:, NWIN - 1, :], 0.0)
            for a in range(NWIN):
                nt = 128 if a < NWIN - 1 else LASTN
                at = ap_.tile([ATOM_DIM + 1, 128], dt.bfloat16, tag="at")
                nc.sync.dma_start(at[:, 0:nt], atomT_d[:, a * 128:a * 128 + nt])
                ps = psE.tile([128, 512], dt.float32, tag="pse", name="ps")
                nc.tensor.matmul(ps[0:nt, 0:HID], at[:, 0:nt], win[:],
                                 start=True, stop=True)
                nc.scalar.activation(out=fdh[0:nt, a, :], in_=ps[0:nt, 0:HID],
                                     func=AFT.Copy)
                j = a % 8
                if j == 0:
                    fs_sb = sp.tile([128, 8, HID], dt.bfloat16, tag="fs_sb",
                                    name="fs_sb")
                proj_block(0, a, fs_sb, j, a & 1, 2 + (a & 1))
                if j == 7 or a == NWIN - 1:
                    flush_fs(a - j, a + 1, fs_sb)
                    fire_ags((a - j) * 128, min(NPC, (a + 1) * 128), 0)

            # ---- per-layer edge phase; proj(l+1) + its AGs hide inside
            for l in range(LAYERS):
                last = l == LAYERS - 1
                holdover = None
                for g in range(NGRP):
                    c0, c1 = P['group_cols'][g]
                    Cg = c1 - c0
                    wins = list(range(g * G, min(NWIN, g * G + G)))
                    pg = P['pairs'][g]
                    Pg = len(pg)
                    pbase = pg[0]['pi']

                    ohEt = op_.tile([128, Pmax, 128], dt.bfloat16, tag="ohE")
                    ohAt = op_.tile([128, Pmax, 128], dt.bfloat16, tag="ohA")
                    nc.sync.dma_start(ohEt[:, 0:Pg, :],
                                      ohE_d[:, pbase:pbase + Pg, :])
                    nc.sync.dma_start(ohAt[:, 0:Pg, :],
                                      ohA_d[:, pbase:pbase + Pg, :])
                    fsg = wp.tile([128, Cmax, HID], dt.bfloat16, tag="fsg")
                    for (k, soff, n) in P['pieces'][g]:
                        nc.gpsimd.dma_gather(
                            fsg[:, soff // 128 - c0:(soff + n) // 128 - c0, :],
                            fs_q[l % 2][k][:],
                            fsi[:, soff // 16:(soff + n) // 16], n, n, HID)

                    # expand fd per slot-col (2 cols per PSUM tile) + u = fs+fd
                    upre = up_.tile([128, Cmax, HID], dt.bfloat16, tag="upre",
                                    bufs=1)
                    percol = {}
                    for i, pr in enumerate(pg):
                        percol.setdefault(pr['c'], []).append(i)
                    for cl0 in range(0, Cg, 2):
                        ncol = min(2, Cg - cl0)
                        ps = psE.tile([128, 512], dt.float32, tag="pse",
                                      name="ps")
                        for dc in range(ncol):
                            cl = cl0 + dc
                            idxs = percol[cl]
                            for ii, i in enumerate(idxs):
                                pr = pg[i]
                                nc.tensor.matmul(
                                    ps[:, dc * HID:(dc + 1) * HID],
                                    ohEt[:, i, :], fdh[:, wins[pr['w']], :],
                                    start=(ii == 0), stop=(ii == len(idxs) - 1))
                        nc.vector.tensor_tensor(
                            out=upre[:, cl0:cl0 + ncol, :],
                            in0=ps[:, 0:ncol * HID].rearrange(
                                "p (t e) -> p t e", t=ncol),
                            in1=fsg[:, cl0:cl0 + ncol, :],
                            op=mybir.AluOpType.add)

                    # alpha-folded prelu + per-head logits
                    for h in range(HEADS):
                        kp = int(pos_cnt[l, h])
                        if kp:
                            nc.scalar.activation(
                                out=upre[:, 0:Cg, h * OUT:h * OUT + kp],
                                in_=upre[:, 0:Cg, h * OUT:h * OUT + kp],
                                func=AFT.Prelu, alpha=ALPHA)
                        if kp < OUT:
                            nc.scalar.activation(
                                out=upre[:, 0:Cg, h * OUT + kp:(h + 1) * OUT],
                                in_=upre[:, 0:Cg, h * OUT + kp:(h + 1) * OUT],
                                func=AFT.Prelu, alpha=1.0 / ALPHA)
                    lg = mp.tile([128, Cmax, 4], dt.float32, tag="lg")
                    for h in range(HEADS):
                        nc.vector.tensor_reduce(
                            out=lg[:, 0:Cg, h],
                            in_=upre[:, 0:Cg, h * OUT:(h + 1) * OUT],
                            axis=mybir.AxisListType.X, op=mybir.AluOpType.add)
                    pay = yp.tile([128, Cmax, HID + 4], dt.bfloat16, tag="pay")
                    nc.scalar.activation(out=pay[:, 0:Cg, HID:HID + 4],
                                         in_=lg[:, 0:Cg, :], func=AFT.Exp)
                    nc.vector.tensor_tensor(
                        out=pay[:, 0:Cg, 0:HID].rearrange(
                            "p t (h d) -> p t h d", h=HEADS),
                        in0=fsg[:, 0:Cg, :].rearrange(
                            "p t (h d) -> p t h d", h=HEADS),
                        in1=pay[:, 0:Cg, HID:HID + 4].unsqueeze(3).broadcast_to(
                            [128, Cg, HEADS, OUT]),
                        op=mybir.AluOpType.mult)

                    if holdover is not None:
                        _agg_norm(nc, mybir, aggp, mp, sp, fdh, out_d,
                                  holdover, last, proj_block, flush_fs,
                                  fire_ags, l)
                    holdover = (g, wins, pg, ohAt, pay)
                if holdover is not None:
                    _agg_norm(nc, mybir, aggp, mp, sp, fdh, out_d,
                              holdover, last, proj_block, flush_fs,
                              fire_ags, l)
    nc.compile()
    return nc


def _agg_norm(nc, mybir, aggp, mp, sp, fdh, out_d, holdover, last,
              proj_block, flush_fs, fire_ags, l):
    dt = mybir.dt
    AFT = mybir.ActivationFunctionType
    g, wins, pg, ohAt, pay = holdover
    cur = {}
    outst = None
    fs_sb = None
    if last:
        outst = sp.tile([128, G, HID], dt.float32, tag="outst", bufs=1)
    else:
        fs_sb = sp.tile([128, G, HID], dt.bfloat16, tag="fs_sb", name="fs_sb")
    for i, pr in enumerate(pg):
        wl = pr['w']
        if pr['a_start']:
            cur[wl] = aggp.tile([128, HID + 4], dt.float32, tag=f"agg{wl}",
                                name=f"aggps{wl}")
        psA = cur[wl]
        nc.tensor.matmul(psA[:, 0:HID + 4], ohAt[:, i, :],
                         pay[:, pr['c'], 0:HID + 4],
                         start=pr['a_start'], stop=pr['a_stop'])
        if pr['a_stop']:
            W = wins[wl]
            denf = mp.tile([128, 4], dt.float32, tag="denf")
            rec = mp.tile([128, 4], dt.float32, tag="rec")
            nc.scalar.activation(out=denf[:], in_=psA[:, HID:HID + 4],
                                 func=AFT.Copy, bias=EPS)
            nc.vector.reciprocal(out=rec[:], in_=denf[:])
            for h in range(4):
                dst = (outst[:, wl, h * OUT:(h + 1) * OUT] if last
                       else fdh[:, W, h * OUT:(h + 1) * OUT])
                nc.scalar.activation(
                    out=dst, in_=psA[:, h * OUT:(h + 1) * OUT],
                    func=AFT.Copy, scale=rec[:, h:h + 1])
            if not last:
                # project window W for layer l+1 (hides proj + AG in edge)
                proj_block(l + 1, W, fs_sb, wl, wl, wl)
    w0, w1 = wins[0], wins[-1] + 1
    if last:
        LASTN = NPC - 128 * (NWIN - 1)
        fullw = len(wins) if wins[-1] < NWIN - 1 else len(wins) - 1
        if fullw:
            nc.sync.dma_start(
                out_d[w0 * 128:(w0 + fullw) * 128, :].rearrange(
                    "(a p) e -> p a e", p=128),
                outst[:, 0:fullw, :])
        if wins[-1] == NWIN - 1:
            nc.sync.dma_start(
                out_d[(NWIN - 1) * 128:NPC, :],
                outst[0:LASTN, len(wins) - 1, :])
    else:
        flush_fs(w0, w1, fs_sb)
        fire_ags(w0 * 128, min(NPC, w1 * 128), (l + 1) % 2)


# ---------------------------------------------------------------------------


def kernel(**inputs):
    from concourse.bass_utils import run_bass_kernel_spmd

    src = np.asarray(inputs['src'])
    dst = np.asarray(inputs['dst'])
    atom = np.asarray(inputs['atom_feat']).astype(np.float32)
    Ws_eff, Wd_eff, bs_eff, bd_eff, pos_cnt, zero_cnt, T2inv = _fold_weights(
        inputs['W_src'], inputs['b_src'], inputs['W_dst'], inputs['b_dst'],
        inputs['attn'], inputs['bias'])
    for l in range(LAYERS):
        assert np.abs(bs_eff[l]).max() < 1e-12 and np.abs(bd_eff[l]).max() < 1e-12, \
            "nonzero GAT biases not supported by this kernel build"
    assert (zero_cnt == 0).all(), "zero attention weights not supported"

    P = _prep(src, dst)

    win_np = np.zeros((ATOM_DIM + 1, HID), np.float32)
    win_np[:ATOM_DIM] = np.asarray(inputs['W_in'])
    win_np[ATOM_DIM] = np.asarray(inputs['b_in'])
    wsd_np = np.zeros((128, 2 * LAYERS, 512), np.float32)
    for l in range(LAYERS):
        for kc in range(2):
            wsd_np[:, l * 2 + kc, 0:HID] = Ws_eff[l][kc * 128:(kc + 1) * 128]
            wsd_np[:, l * 2 + kc, HID:512] = Wd_eff[l][kc * 128:(kc + 1) * 128]

    nc = _build(P, pos_cnt)

    in_maps = []
    for c in range(NCORES):
        cd = P['cores'][c]
        at = np.zeros((ATOM_DIM + 1, NPC), np.float32)
        at[:ATOM_DIM] = atom[c * NPC + P['nodes_by_r'][c]].T
        at[ATOM_DIM] = 1.0
        in_maps.append({
            'atomT': _bf(at), 'win': _bf(win_np), 'wsd': _bf(wsd_np),
            'fsi': _wrap16(cd['fs_idx']),
            'ohE': cd['ohE'], 'ohA': cd['ohA'],
            'ident': _bf(np.eye(128, dtype=np.float32)),
        })
    res = run_bass_kernel_spmd(nc, in_maps, core_ids=list(range(NCORES)),
                               trace=bool(os.environ.get('KBT_TRACE')))
    kernel._last = res
    full = np.empty((N, HID), np.float64)
    for c in range(NCORES):
        full[c * NPC + P['nodes_by_r'][c]] = res.results[c]['out']
    full = full @ T2inv + np.asarray(inputs['bias'])[LAYERS - 1][None]
    return full.astype(np.float32)


if __name__ == '__main__':
    import jax
    with jax.default_device(jax.devices('cpu')[0]):
        import reference
        inputs = {k: np.asarray(v) for k, v in reference.setup_inputs().items()}
    got = kernel(**inputs)
    print("kernel out:", got.shape, got.dtype, np.abs(got).mean())


# revision 13
# speedup vs baseline: 1.5930x; 1.0250x over previous
"""Trainium2 8-core GATv2 message-passing kernel (nn_AtomGraphEncoder).

Design (v3 — PE-onehot, quarter-split pipelined AllGather):
- Nodes block-sharded 8x12500, degree-balanced permutation into 98 windows
  of 128 nodes per core; edges assigned to dst's core.
- fs table split into 4 quarter tensors (one per 3125-row slice of every
  core); each quarter AllGathered as soon as projection covers it, so the
  collective pipelines behind the projection and the first edge gathers.
- Per layer, per group of 6 windows: dma_gather fs rows (the only
  GPSIMD-heavy op), expand fd per edge-slot via one-hot matmuls (PE),
  alpha-folded prelu (ACT) + head reduces (DVE) for GATv2 logits, exp,
  msg = ex*fs, segment-sum aggregation via transposed one-hot matmuls
  into per-window PSUM (PE) — no dma_scatter_add, no HBM accumulator.
- |a| and the 0.2 lrelu factor folded into W columns on host
  (pos: a, neg: -0.2|a| with alpha=5 prelu), undone on output.
"""
import sys
import os

import numpy as np
import ml_dtypes

sys.path.insert(0, '/opt/trn_rl_repo')

N, E = 100000, 400000
ATOM_DIM, HID, LAYERS, HEADS = 74, 256, 3, 4
OUT = HID // HEADS
NCORES = 8
NPC = N // NCORES            # 12500
NPQ = NPC // 4               # 3125 rows per core per quarter
QTR = NPQ * NCORES           # 25000 rows per quarter table
CLS = 4
NWIN = (NPC + 127) // 128    # 98
G = 6
NGRP = (NWIN + G - 1) // G   # 17
BF = ml_dtypes.bfloat16
EPS = 1e-20
ALPHA = 0.2

# ---------------------------------------------------------------------------
# host prep


def _fold_weights(W_src, b_src, W_dst, b_dst, attn, bias):
    Ts, Tinvs = [], []
    pos_cnt = np.zeros((LAYERS, HEADS), np.int64)
    zero_cnt = np.zeros((LAYERS, HEADS), np.int64)
    for l in range(LAYERS):
        Tl = np.zeros((HID, HID), np.float64)
        Tinv = np.zeros((HID, HID), np.float64)
        for h in range(HEADS):
            a = np.asarray(attn)[l, h].astype(np.float64)
            order = np.concatenate([
                np.where(a > 0)[0], np.where(a == 0)[0], np.where(a < 0)[0]])
            pos_cnt[l, h] = (a > 0).sum()
            zero_cnt[l, h] = (a == 0).sum()
            for j, p in enumerate(order):
                if a[p] > 0:
                    s = a[p]
                elif a[p] == 0:
                    s = 1.0
                else:
                    s = -ALPHA * abs(a[p])
                Tl[h * OUT + p, h * OUT + j] = s
                Tinv[h * OUT + j, h * OUT + p] = 1.0 / s
        Ts.append(Tl)
        Tinvs.append(Tinv)
    Ws_eff, Wd_eff, bs_eff, bd_eff = [], [], [], []
    for l in range(LAYERS):
        Tp = np.eye(HID) if l == 0 else Tinvs[l - 1]
        Ws = np.asarray(W_src)[l].astype(np.float64)
        Wd = np.asarray(W_dst)[l].astype(np.float64)
        bprev = np.zeros(HID) if l == 0 else np.asarray(bias)[l - 1].astype(np.float64)
        Ws_eff.append((Tp @ Ws @ Ts[l]).astype(np.float32))
        Wd_eff.append((Tp @ Wd @ Ts[l]).astype(np.float32))
        bs_eff.append(((np.asarray(b_src)[l] + bprev @ Ws) @ Ts[l]).astype(np.float32))
        bd_eff.append(((np.asarray(b_dst)[l] + bprev @ Wd) @ Ts[l]).astype(np.float32))
    return Ws_eff, Wd_eff, bs_eff, bd_eff, pos_cnt, zero_cnt, Tinvs[-1]


def _balanced_perm(deg):
    caps = np.full(NWIN, 128, np.int64)
    caps[-1] = NPC - 128 * (NWIN - 1)
    order = np.argsort(-deg, kind="stable")
    fill = np.zeros(NWIN, np.int64)
    r_of_node = np.empty(NPC, np.int64)
    seq = np.concatenate([np.arange(NWIN), np.arange(NWIN)[::-1]])
    ptr = 0
    for node in order:
        while fill[seq[ptr % (2 * NWIN)]] >= caps[seq[ptr % (2 * NWIN)]]:
            ptr += 1
        w = seq[ptr % (2 * NWIN)]
        r_of_node[node] = w * 128 + fill[w]
        fill[w] += 1
        ptr += 1
    nodes_by_r = np.empty(NPC, np.int64)
    nodes_by_r[r_of_node] = np.arange(NPC)
    return r_of_node, nodes_by_r


def _prep(src, dst):
    src = np.asarray(src).astype(np.int64)
    dst = np.asarray(dst).astype(np.int64)

    cores_edges = []
    r_of_node_all = []
    nodes_by_r_all = []
    for c in range(NCORES):
        m = (dst >= c * NPC) & (dst < (c + 1) * NPC)
        es, dl = src[m], dst[m] - c * NPC
        deg = np.bincount(dl, minlength=NPC)
        r_of_node, nodes_by_r = _balanced_perm(deg)
        cores_edges.append((es, r_of_node[dl]))
        r_of_node_all.append(r_of_node)
        nodes_by_r_all.append(nodes_by_r)

    # quarter table row for each absolute node id:
    # class = r//NPQ, idx in table = core*NPQ + r%NPQ
    fscls = np.empty(N, np.int64)
    fsidx = np.empty(N, np.int64)
    for c in range(NCORES):
        r = r_of_node_all[c]
        fscls[c * NPC:(c + 1) * NPC] = r // NPQ
        fsidx[c * NPC:(c + 1) * NPC] = c * NPQ + (r % NPQ)

    cnt = np.zeros((NCORES, NWIN, CLS), np.int64)
    core_wk = []
    for c in range(NCORES):
        es, r_d = cores_edges[c]
        w = r_d // 128
        k = fscls[es]
        np.add.at(cnt[c], (w, k), 1)
        core_wk.append((w, k))
    seg = cnt.max(axis=0)

    run_base = np.zeros((NWIN, CLS), np.int64)
    pieces = [[] for _ in range(NGRP)]
    group_cols = []
    off = 0
    for g in range(NGRP):
        wins = list(range(g * G, min(NWIN, g * G + G)))
        g0 = off
        for k in range(CLS):
            cstart = off
            for w in wins:
                run_base[w, k] = off
                off += seg[w, k]
            off += (-(off - cstart)) % 128
            if off > cstart:
                pieces[g].append((k, cstart, off - cstart))
        group_cols.append((g0 // 128, off // 128))
    S = off
    T_all = S // 128

    pairs = [[] for _ in range(NGRP)]
    PM = np.full((T_all, G), -1, np.int64)
    pi = 0
    for g in range(NGRP):
        wins = list(range(g * G, min(NWIN, g * G + G)))
        c0, c1 = group_cols[g]
        plist = []
        for col in range(c0, c1):
            a, b = col * 128, (col + 1) * 128
            for wl, w in enumerate(wins):
                for k in range(CLS):
                    lo = max(run_base[w, k], a)
                    hi = min(run_base[w, k] + seg[w, k], b)
                    if lo < hi:
                        plist.append((col - c0, wl))
                        break
        seen = set()
        plist2 = []
        for p in plist:
            if p not in seen:
                seen.add(p)
                plist2.append(p)
        wl_first, wl_last, col_first, col_last = {}, {}, {}, {}
        for i, (cl, wl) in enumerate(plist2):
            wl_first.setdefault(wl, i)
            wl_last[wl] = i
            col_first.setdefault(cl, i)
            col_last[cl] = i
        for i, (cl, wl) in enumerate(plist2):
            pairs[g].append(dict(
                c=cl, w=wl, pi=pi,
                e_start=(col_first[cl] == i), e_stop=(col_last[cl] == i),
                a_start=(wl_first[wl] == i), a_stop=(wl_last[wl] == i)))
            PM[c0 + cl, wl] = pi
            pi += 1
        for w in range(len(wins)):
            assert w in wl_first, f"window {g * G + w} has no pairs"
    P_total = pi
    Pmax = max(len(p) for p in pairs)
    Cmax = max(c1 - c0 for c0, c1 in group_cols)

    cores = []
    for c in range(NCORES):
        es, r_d = cores_edges[c]
        w, k = core_wk[c]
        order = np.lexsort((r_d, k, w))
        es_s, rd_s, w_s, k_s = es[order], r_d[order], w[order], k[order]
        key = w_s * CLS + k_s
        uniq, starts = np.unique(key, return_index=True)
        rank = np.arange(len(key)) - np.repeat(
            starts, np.diff(np.concatenate([starts, [len(key)]])))
        slot = run_base[w_s, k_s] + rank
        fs_idx = np.zeros(S, np.int64)
        fs_idx[slot] = fsidx[es_s]
        g_s = w_s // G
        wl_s = w_s - g_s * G
        pi_e = PM[slot // 128, wl_s]
        assert (pi_e >= 0).all()
        ohA = np.zeros((128, P_total, 128), BF)
        ohE = np.zeros((128, P_total, 128), BF)
        ohA[slot % 128, pi_e, rd_s % 128] = 1.0
        ohE[rd_s % 128, pi_e, slot % 128] = 1.0
        cores.append(dict(fs_idx=fs_idx, ohA=ohA, ohE=ohE))

    return dict(seg=seg, run_base=run_base, pieces=pieces, pairs=pairs,
                group_cols=group_cols, S=S, T_all=T_all, P_total=P_total,
                Pmax=Pmax, Cmax=Cmax, cores=cores,
                nodes_by_r=nodes_by_r_all)


def _bf(x):
    return np.asarray(x).astype(BF)


def _wrap16(idx):
    w = np.ascontiguousarray(np.asarray(idx).reshape(-1, 16).T).astype(np.int16)
    return np.tile(w, (8, 1))


# ---------------------------------------------------------------------------
# bass build


def _build(P, pos_cnt):
    import concourse.bass as bass
    import concourse.tile as tile
    from concourse import bacc, mybir, library_config

    S = P['S']
    Pmax, Cmax, P_total = P['Pmax'], P['Cmax'], P['P_total']

    nc = bacc.Bacc("TRN2", target_bir_lowering=False, debug=False,
                   num_devices=NCORES)
    dt = mybir.dt
    atomT_d = nc.dram_tensor("atomT", [ATOM_DIM + 1, NPC], dt.bfloat16,
                             kind="ExternalInput")
    win_d = nc.dram_tensor("win", [ATOM_DIM + 1, HID], dt.bfloat16,
                           kind="ExternalInput")
    wsd_d = nc.dram_tensor("wsd", [128, 2 * LAYERS, 512], dt.bfloat16,
                           kind="ExternalInput")
    fsi_d = nc.dram_tensor("fsi", [128, S // 16], dt.int16, kind="ExternalInput")
    ohE_d = nc.dram_tensor("ohE", [128, P_total, 128], dt.bfloat16,
                           kind="ExternalInput")
    ohA_d = nc.dram_tensor("ohA", [128, P_total, 128], dt.bfloat16,
                           kind="ExternalInput")
    ident_d = nc.dram_tensor("ident", [128, 128], dt.bfloat16,
                             kind="ExternalInput")
    out_d = nc.dram_tensor("out", [NPC, HID], dt.float32, kind="ExternalOutput")

    fs_bounce = nc.dram_tensor("fs_bounce", [NPC, HID], dt.bfloat16)
    # double-buffered across layers: AG for layer l+1 streams during edge(l)
    fs_q = [[nc.dram_tensor(f"fs_q{b}_{k}", [QTR, HID], dt.bfloat16,
                            addr_space="Shared") for k in range(CLS)]
            for b in range(2)]

    LASTN = NPC - 128 * (NWIN - 1)   # 84
    AFT = mybir.ActivationFunctionType

    with tile.TileContext(nc) as tc:
        nc.gpsimd.load_library(library_config.mlp)
        with tc.tile_pool(name="persist", bufs=1) as pp, \
             tc.tile_pool(name="atp", bufs=2) as ap_, \
             tc.tile_pool(name="htp", bufs=3) as hp, \
             tc.tile_pool(name="stage", bufs=2) as sp, \
             tc.tile_pool(name="ohp", bufs=2) as op_, \
             tc.tile_pool(name="fsgp", bufs=2) as wp, \
             tc.tile_pool(name="upp", bufs=2) as up_, \
             tc.tile_pool(name="payp", bufs=2) as yp, \
             tc.tile_pool(name="smallp", bufs=2) as mp, \
             tc.tile_pool(name="psE", bufs=2, space="PSUM") as psE, \
             tc.tile_pool(name="aggp", bufs=1, space="PSUM") as aggp:

            fsi = pp.tile([128, S // 16], dt.int16, tag="fsi")
            wsd = pp.tile([128, 2 * LAYERS, 512], dt.bfloat16, tag="wsd")
            win = pp.tile([ATOM_DIM + 1, HID], dt.bfloat16, tag="win")
            fdh = pp.tile([128, NWIN, HID], dt.bfloat16, tag="fdh")
            ident = pp.tile([128, 128], dt.bfloat16, tag="ident")
            nc.sync.dma_start(ident[:], ident_d[:])
            nc.sync.dma_start(fsi[:], fsi_d[:])
            nc.sync.dma_start(wsd[:], wsd_d[:])
            nc.sync.dma_start(win[:], win_d[:])

            def fire_ags(lo_row, hi_row, lset):
                """AllGather any fs quarter fully covered in [lo_row, hi_row)."""
                for q in range(CLS):
                    if lo_row < (q + 1) * NPQ <= hi_row:
                        nc.gpsimd.collective_compute(
                            "AllGather", mybir.AluOpType.bypass,
                            replica_groups=[list(range(NCORES))],
                            ins=[fs_bounce[q * NPQ:(q + 1) * NPQ, :].opt()],
                            outs=[fs_q[lset][q][:].opt()])

            def proj_block(l, a, fs_sb, j, ptag0, ptag1):
                """Project window a of layer l: fdh h -> (fs staging, fdh fd)."""
                nt = 128 if a < NWIN - 1 else LASTN
                hTst = hp.tile([128, 2, 128], dt.bfloat16, tag="hTst",
                               name="hTst")
                for cch in range(2):
                    tg = ptag0 if cch == 0 else ptag1
                    pt = aggp.tile([128, 128], dt.bfloat16,
                                   tag=f"agg{tg}", name=f"tp{tg}")
                    nc.tensor.transpose(
                        pt[:], fdh[:, a, cch * 128:(cch + 1) * 128], ident[:])
                    if cch == 0:
                        nc.scalar.activation(out=hTst[:, cch, :], in_=pt[:],
                                             func=AFT.Copy)
                    else:
                        nc.vector.tensor_copy(out=hTst[:, cch, :], in_=pt[:])
                ps = psE.tile([128, 512], dt.float32, tag="pse", name="ps")
                for kc in range(2):
                    nc.tensor.matmul(ps[0:nt, :], hTst[:, kc, 0:nt],
                                     wsd[:, l * 2 + kc, :],
                                     start=(kc == 0), stop=(kc == 1))
                nc.scalar.activation(out=fs_sb[0:nt, j, :], in_=ps[0:nt, 0:HID],
                                     func=AFT.Copy)
                nc.vector.tensor_copy(out=fdh[0:nt, a, :], in_=ps[0:nt, HID:512])

            def flush_fs(a0, a1, fs_sb):
                """DMA staged fs windows [a0, a1) to fs_bounce."""
                nw = a1 - a0
                fullw = nw if a1 - 1 < NWIN - 1 else nw - 1
                if fullw:
                    nc.sync.dma_start(
                        fs_bounce[a0 * 128:(a0 + fullw) * 128, :].rearrange(
                            "(a p) e -> p a e", p=128),
                        fs_sb[:, 0:fullw, :])
                if fullw < nw:
                    nc.sync.dma_start(
                        fs_bounce[(NWIN - 1) * 128:NPC, :],
                        fs_sb[0:LASTN, nw - 1, :])

            # ---- input projection fused with layer-0 projection per block
            nc.vector.memset(fdh[:, NWIN - 1, :], 0.0)
            for a in range(NWIN):
                nt = 128 if a < NWIN - 1 else LASTN
                at = ap_.tile([ATOM_DIM + 1, 128], dt.bfloat16, tag="at")
                nc.sync.dma_start(at[:, 0:nt], atomT_d[:, a * 128:a * 128 + nt])
                itg = 4 + (a & 1)
                ps = aggp.tile([128, HID], dt.float32, tag=f"agg{itg}",
                               name=f"ips{itg}")
                nc.tensor.matmul(ps[0:nt, 0:HID], at[:, 0:nt], win[:],
                                 start=True, stop=True)
                nc.scalar.activation(out=fdh[0:nt, a, :], in_=ps[0:nt, 0:HID],
                                     func=AFT.Copy)
                j = a % 8
                if j == 0:
                    fs_sb = sp.tile([128, 8, HID], dt.bfloat16, tag="fs_sb",
                                    name="fs_sb")
                proj_block(0, a, fs_sb, j, a & 1, 2 + (a & 1))
                if j == 7 or a == NWIN - 1:
                    flush_fs(a - j, a + 1, fs_sb)
                    fire_ags((a - j) * 128, min(NPC, (a + 1) * 128), 0)

            # ---- per-layer edge phase; proj(l+1) + its AGs hide inside
            for l in range(LAYERS):
                last = l == LAYERS - 1
                holdover = None
                for g in range(NGRP):
                    c0, c1 = P['group_cols'][g]
                    Cg = c1 - c0
                    wins = list(range(g * G, min(NWIN, g * G + G)))
                    pg = P['pairs'][g]
                    Pg = len(pg)
                    pbase = pg[0]['pi']

                    ohEt = op_.tile([128, Pmax, 128], dt.bfloat16, tag="ohE")
                    ohAt = op_.tile([128, Pmax, 128], dt.bfloat16, tag="ohA")
                    nc.sync.dma_start(ohEt[:, 0:Pg, :],
                                      ohE_d[:, pbase:pbase + Pg, :])
                    nc.sync.dma_start(ohAt[:, 0:Pg, :],
                                      ohA_d[:, pbase:pbase + Pg, :])
                    fsg = wp.tile([128, Cmax, HID], dt.bfloat16, tag="fsg")
                    for (k, soff, n) in P['pieces'][g]:
                        nc.gpsimd.dma_gather(
                            fsg[:, soff // 128 - c0:(soff + n) // 128 - c0, :],
                            fs_q[l % 2][k][:],
                            fsi[:, soff // 16:(soff + n) // 16], n, n, HID)

                    # expand fd per slot-col (2 cols per PSUM tile) + u = fs+fd
                    upre = up_.tile([128, Cmax, HID], dt.bfloat16, tag="upre",
                                    bufs=1)
                    percol = {}
                    for i, pr in enumerate(pg):
                        percol.setdefault(pr['c'], []).append(i)
                    for cl0 in range(0, Cg, 2):
                        ncol = min(2, Cg - cl0)
                        ps = psE.tile([128, 512], dt.float32, tag="pse",
                                      name="ps")
                        for dc in range(ncol):
                            cl = cl0 + dc
                            idxs = percol[cl]
                            for ii, i in enumerate(idxs):
                                pr = pg[i]
                                nc.tensor.matmul(
                                    ps[:, dc * HID:(dc + 1) * HID],
                                    ohEt[:, i, :], fdh[:, wins[pr['w']], :],
                                    start=(ii == 0), stop=(ii == len(idxs) - 1))
                        nc.vector.tensor_tensor(
                            out=upre[:, cl0:cl0 + ncol, :],
                            in0=ps[:, 0:ncol * HID].rearrange(
                                "p (t e) -> p t e", t=ncol),
                            in1=fsg[:, cl0:cl0 + ncol, :],
                            op=mybir.AluOpType.add)

                    # alpha-folded prelu + per-head logits
                    for h in range(HEADS):
                        kp = int(pos_cnt[l, h])
                        if kp:
                            nc.scalar.activation(
                                out=upre[:, 0:Cg, h * OUT:h * OUT + kp],
                                in_=upre[:, 0:Cg, h * OUT:h * OUT + kp],
                                func=AFT.Prelu, alpha=ALPHA)
                        if kp < OUT:
                            nc.scalar.activation(
                                out=upre[:, 0:Cg, h * OUT + kp:(h + 1) * OUT],
                                in_=upre[:, 0:Cg, h * OUT + kp:(h + 1) * OUT],
                                func=AFT.Prelu, alpha=1.0 / ALPHA)
                    lg = mp.tile([128, Cmax, 4], dt.float32, tag="lg")
                    for h in range(HEADS):
                        nc.vector.tensor_reduce(
                            out=lg[:, 0:Cg, h],
                            in_=upre[:, 0:Cg, h * OUT:(h + 1) * OUT],
                            axis=mybir.AxisListType.X, op=mybir.AluOpType.add)
                    pay = yp.tile([128, Cmax, HID + 4], dt.bfloat16, tag="pay")
                    nc.scalar.activation(out=pay[:, 0:Cg, HID:HID + 4],
                                         in_=lg[:, 0:Cg, :], func=AFT.Exp)
                    nc.vector.tensor_tensor(
                        out=pay[:, 0:Cg, 0:HID].rearrange(
                            "p t (h d) -> p t h d", h=HEADS),
                        in0=fsg[:, 0:Cg, :].rearrange(
                            "p t (h d) -> p t h d", h=HEADS),
                        in1=pay[:, 0:Cg, HID:HID + 4].unsqueeze(3).broadcast_to(
                            [128, Cg, HEADS, OUT]),
                        op=mybir.AluOpType.mult)

                    if holdover is not None:
                        _agg_norm(nc, mybir, aggp, mp, sp, fdh, out_d,
                                  holdover, last, proj_block, flush_fs,
                                  fire_ags, l)
                    holdover = (g, wins, pg, ohAt, pay)
                if holdover is not None:
                    _agg_norm(nc, mybir, aggp, mp, sp, fdh, out_d,
                              holdover, last, proj_block, flush_fs,
                              fire_ags, l)
    nc.compile()
    return nc


def _agg_norm(nc, mybir, aggp, mp, sp, fdh, out_d, holdover, last,
              proj_block, flush_fs, fire_ags, l):
    dt = mybir.dt
    AFT = mybir.ActivationFunctionType
    g, wins, pg, ohAt, pay = holdover
    cur = {}
    outst = None
    fs_sb = None
    if last:
        outst = sp.tile([128, G, HID], dt.float32, tag="outst", bufs=1)
    else:
        fs_sb = sp.tile([128, G, HID], dt.bfloat16, tag="fs_sb", name="fs_sb")
    for i, pr in enumerate(pg):
        wl = pr['w']
        if pr['a_start']:
            cur[wl] = aggp.tile([128, HID + 4], dt.float32, tag=f"agg{wl}",
                                name=f"aggps{wl}")
        psA = cur[wl]
        nc.tensor.matmul(psA[:, 0:HID + 4], ohAt[:, i, :],
                         pay[:, pr['c'], 0:HID + 4],
                         start=pr['a_start'], stop=pr['a_stop'])
        if pr['a_stop']:
            W = wins[wl]
            denf = mp.tile([128, 4], dt.float32, tag="denf")
            rec = mp.tile([128, 4], dt.float32, tag="rec")
            nc.scalar.activation(out=denf[:], in_=psA[:, HID:HID + 4],
                                 func=AFT.Copy, bias=EPS)
            nc.vector.reciprocal(out=rec[:], in_=denf[:])
            for h in range(4):
                dst = (outst[:, wl, h * OUT:(h + 1) * OUT] if last
                       else fdh[:, W, h * OUT:(h + 1) * OUT])
                nc.scalar.activation(
                    out=dst, in_=psA[:, h * OUT:(h + 1) * OUT],
                    func=AFT.Copy, scale=rec[:, h:h + 1])
            if not last:
                # project window W for layer l+1 (hides proj + AG in edge)
                proj_block(l + 1, W, fs_sb, wl, wl, wl)
    w0, w1 = wins[0], wins[-1] + 1
    if last:
        LASTN = NPC - 128 * (NWIN - 1)
        fullw = len(wins) if wins[-1] < NWIN - 1 else len(wins) - 1
        if fullw:
            nc.sync.dma_start(
                out_d[w0 * 128:(w0 + fullw) * 128, :].rearrange(
                    "(a p) e -> p a e", p=128),
                outst[:, 0:fullw, :])
        if wins[-1] == NWIN - 1:
            nc.sync.dma_start(
                out_d[(NWIN - 1) * 128:NPC, :],
                outst[0:LASTN, len(wins) - 1, :])
    else:
        flush_fs(w0, w1, fs_sb)
        fire_ags(w0 * 128, min(NPC, w1 * 128), (l + 1) % 2)


# ---------------------------------------------------------------------------


def kernel(**inputs):
    from concourse.bass_utils import run_bass_kernel_spmd

    src = np.asarray(inputs['src'])
    dst = np.asarray(inputs['dst'])
    atom = np.asarray(inputs['atom_feat']).astype(np.float32)
    Ws_eff, Wd_eff, bs_eff, bd_eff, pos_cnt, zero_cnt, T2inv = _fold_weights(
        inputs['W_src'], inputs['b_src'], inputs['W_dst'], inputs['b_dst'],
        inputs['attn'], inputs['bias'])
    for l in range(LAYERS):
        assert np.abs(bs_eff[l]).max() < 1e-12 and np.abs(bd_eff[l]).max() < 1e-12, \
            "nonzero GAT biases not supported by this kernel build"
    assert (zero_cnt == 0).all(), "zero attention weights not supported"

    P = _prep(src, dst)

    win_np = np.zeros((ATOM_DIM + 1, HID), np.float32)
    win_np[:ATOM_DIM] = np.asarray(inputs['W_in'])
    win_np[ATOM_DIM] = np.asarray(inputs['b_in'])
    wsd_np = np.zeros((128, 2 * LAYERS, 512), np.float32)
    for l in range(LAYERS):
        for kc in range(2):
            wsd_np[:, l * 2 + kc, 0:HID] = Ws_eff[l][kc * 128:(kc + 1) * 128]
            wsd_np[:, l * 2 + kc, HID:512] = Wd_eff[l][kc * 128:(kc + 1) * 128]

    nc = _build(P, pos_cnt)

    in_maps = []
    for c in range(NCORES):
        cd = P['cores'][c]
        at = np.zeros((ATOM_DIM + 1, NPC), np.float32)
        at[:ATOM_DIM] = atom[c * NPC + P['nodes_by_r'][c]].T
        at[ATOM_DIM] = 1.0
        in_maps.append({
            'atomT': _bf(at), 'win': _bf(win_np), 'wsd': _bf(wsd_np),
            'fsi': _wrap16(cd['fs_idx']),
            'ohE': cd['ohE'], 'ohA': cd['ohA'],
            'ident': _bf(np.eye(128, dtype=np.float32)),
        })
    res = run_bass_kernel_spmd(nc, in_maps, core_ids=list(range(NCORES)),
                               trace=bool(os.environ.get('KBT_TRACE')))
    kernel._last = res
    full = np.empty((N, HID), np.float64)
    for c in range(NCORES):
        full[c * NPC + P['nodes_by_r'][c]] = res.results[c]['out']
    full = full @ T2inv + np.asarray(inputs['bias'])[LAYERS - 1][None]
    return full.astype(np.float32)


if __name__ == '__main__':
    import jax
    with jax.default_device(jax.devices('cpu')[0]):
        import reference
        inputs = {k: np.asarray(v) for k, v in reference.setup_inputs().items()}
    got = kernel(**inputs)
    print("kernel out:", got.shape, got.dtype, np.abs(got).mean())
